# revision 1
# baseline (speedup 1.0000x reference)
# kernel.py — CrystalGCNEncoder (3-layer GAT + global attention pooling) on 8 trn2
# NeuronCores.  Graph-sharded: each core owns 25 graphs' nodes (slots, balanced by
# in-degree over 21 tiles of 128) and all edges whose dst lives there.  Device work
# is split into small SPMD launches; the host only restructures (shard / pad /
# transpose / concat) between launches:
#   P(l):  feat_l = x_l @ [W|W@al] and er_l = x_l @ (W@ar) for own slots (matmuls)
#   L(l):  per-edge gather of feat rows (el bundled in the row tail), edge softmax
#          without max-subtraction (logits are O(1); max cancels exactly), one-hot
#          matmul aggregation in PSUM, normalize + bias + ELU -> x_{l+1}
#   POOL:  gate MLP, per-graph softmax via graph-one-hot matmuls, fp32 latent heads
import numpy as np
import ml_dtypes

N, E, G = 20000, 320000, 200
F_IN, HID, H, LAT = 128, 128, 4, 128
O1, O2, O3 = HID // 2, HID, 2 * HID
D1, D2, D3 = H * O1, H * O2, H * O3          # 256, 512, 1024
NEG_SLOPE = 0.2
NCORES = 8
BF16 = ml_dtypes.bfloat16


def _row_elems(d):          # feat row: [d feats | 4 el | pad] bf16, 256B-multiple
    b = (d + 4) * 2
    return ((b + 255) // 256 * 256) // 2


class Cfg:
    def __init__(self, n, e, g, ntiles, cpt, ncores=NCORES):
        self.n, self.e, self.g, self.ncores = n, e, g, ncores
        self.gpc = g // ncores
        self.ntiles = ntiles
        self.nloc = ntiles * 128
        self.nstar = self.nloc * ncores
        self.cpt = cpt
        self.tpe = cpt * 128
        self.eloc = ntiles * self.tpe
        self.nch = self.eloc // 128
        self.gpad = 32


CFG_FULL = Cfg(N, E, G, ntiles=21, cpt=16)


# ------------------------------------------------------------------ host prep
def host_prep(cfg, node_feat, src, dst, graph_ids):
    n, nc_ = cfg.n, cfg.ncores
    node_feat = np.asarray(node_feat, np.float32)
    src = np.asarray(src).astype(np.int64)
    dst = np.asarray(dst).astype(np.int64)
    graph_ids = np.asarray(graph_ids).astype(np.int64)

    gbounds = np.arange(nc_ + 1) * cfg.gpc
    nbounds = np.searchsorted(graph_ids, gbounds)
    core_of_node = np.searchsorted(nbounds, np.arange(n), side="right") - 1
    indeg = np.bincount(dst, minlength=n)

    glob2slot = np.zeros(n, np.int64)
    tile_of_node = np.zeros(n, np.int64)
    slotpos_of_node = np.zeros(n, np.int64)
    for c in range(nc_):
        nodes = np.arange(nbounds[c], nbounds[c + 1])
        assert len(nodes) <= cfg.nloc
        order = nodes[np.argsort(-indeg[nodes], kind="stable")]
        loads = np.zeros(cfg.ntiles, np.int64)
        counts = np.zeros(cfg.ntiles, np.int64)
        for nd in order:
            free = np.nonzero(counts < 128)[0]
            tgt = free[np.argmin(loads[free])]
            tile_of_node[nd] = tgt
            slotpos_of_node[nd] = counts[tgt]
            glob2slot[nd] = c * cfg.nloc + tgt * 128 + counts[tgt]
            counts[tgt] += 1
            loads[tgt] += indeg[nd]
        assert loads.max() <= cfg.tpe

    edge_core = core_of_node[dst]
    idx32_l, oh_l, oht_l, goh_l = [], [], [], []
    for c in range(nc_):
        eids = np.nonzero(edge_core == c)[0]
        assert len(eids) <= cfg.eloc
        src_slot = np.zeros(cfg.eloc, np.int64)
        dst_pos = np.full(cfg.eloc, -1, np.int64)
        et = tile_of_node[dst[eids]]
        for t in range(cfg.ntiles):
            sel = eids[et == t]
            assert len(sel) <= cfg.tpe
            b = t * cfg.tpe
            src_slot[b : b + len(sel)] = glob2slot[src[sel]]
            dst_pos[b : b + len(sel)] = slotpos_of_node[dst[sel]]
        # per-chunk indices [128, nch] (edge i of chunk ch at [i, ch])
        idx32_l.append(np.ascontiguousarray(
            src_slot.reshape(cfg.nch, 128).T).astype(np.int32))
        oh = np.zeros((cfg.eloc, 128), np.float32)
        v = dst_pos >= 0
        oh[np.nonzero(v)[0], dst_pos[v]] = 1.0
        oh_c = oh.reshape(cfg.nch, 128, 128)
        oh_l.append(oh_c.astype(BF16))
        oht_l.append(np.ascontiguousarray(oh_c.transpose(0, 2, 1)).astype(BF16))
        goh = np.zeros((cfg.ntiles, 128, cfg.gpad), np.float32)
        nodes = np.arange(nbounds[c], nbounds[c + 1])
        goh[tile_of_node[nodes], slotpos_of_node[nodes],
            graph_ids[nodes] - c * cfg.gpc] = 1.0
        goh_l.append(goh.astype(BF16))

    x1 = np.zeros((cfg.nstar, F_IN), np.float32)
    x1[glob2slot] = node_feat
    return dict(glob2slot=glob2slot, nbounds=nbounds, idx32=idx32_l,
                oh=oh_l, oht=oht_l, goh=goh_l, x1=x1)


def fold_weights(W, al, ar):
    Din, D = W.shape
    Hh, O = al.shape
    Wl = np.einsum("iho,ho->ih", W.reshape(Din, Hh, O), al)
    Wr = np.einsum("iho,ho->ih", W.reshape(Din, Hh, O), ar)
    return np.concatenate([W, Wl], 1).astype(np.float32), Wr.astype(np.float32)


def xT_own_blocks(cfg, xblk):
    """[nloc, Din] -> [128, (Din/128)*nloc] with block kc at cols kc*nloc+slot."""
    K = xblk.shape[1] // 128
    return np.ascontiguousarray(
        xblk.reshape(cfg.nloc, K, 128).transpose(2, 1, 0).reshape(128, K * cfg.nloc))


def wstack(Waug):
    """[Din, C] -> [128, Din/128, C] (partition-major K chunks)."""
    Din, C = Waug.shape
    return np.ascontiguousarray(Waug.reshape(Din // 128, 128, C).transpose(1, 0, 2))


def _colchunks(c):
    out, s = [], 0
    while s < c:
        w = min(512, c - s)
        out.append((s, w))
        s += w
    return out


# ------------------------------------------------------------------ builders
def build_P(cfg, Din, Dout):
    import concourse.tile as tile
    from concourse import bacc, mybir

    bf = mybir.dt.bfloat16
    K = Din // 128
    ROW = _row_elems(Dout)
    nc = bacc.Bacc("TRN2", target_bir_lowering=False, debug=False,
                   num_devices=cfg.ncores)
    xT = nc.dram_tensor("xT", [128, K * cfg.nloc], bf, kind="ExternalInput").ap()
    Wa = nc.dram_tensor("Wa", [128, K, Dout + 4], bf, kind="ExternalInput").ap()
    Wr = nc.dram_tensor("Wr", [128, K, 4], bf, kind="ExternalInput").ap()
    feat = nc.dram_tensor("feat", [cfg.nloc, ROW], bf, kind="ExternalOutput").ap()
    er = nc.dram_tensor("er", [cfg.nloc, 4], bf, kind="ExternalOutput").ap()
    cks = _colchunks(Dout + 4)
    with tile.TileContext(nc) as tc:
        with tc.tile_pool(name="w", bufs=1) as wp, \
             tc.tile_pool(name="x", bufs=3) as xp, \
             tc.tile_pool(name="ps", bufs=2, space="PSUM") as pp, \
             tc.tile_pool(name="o", bufs=3) as op:
            Wsb = wp.tile([128, K, Dout + 4], bf)
            nc.sync.dma_start(Wsb[:], Wa[:])
            Wrsb = wp.tile([128, K, 4], bf)
            nc.sync.dma_start(Wrsb[:], Wr[:])
            for t in range(cfg.ntiles):
                pa = [pp.tile([128, w], mybir.dt.float32, tag=f"pa{j}", name=f"pa{j}")
                      for j, (s, w) in enumerate(cks)]
                pe = pp.tile([128, 4], mybir.dt.float32, tag="pe")
                for kc in range(K):
                    xt = xp.tile([128, 128], bf)
                    nc.sync.dma_start(
                        xt[:], xT[:, kc * cfg.nloc + t * 128:
                                  kc * cfg.nloc + (t + 1) * 128])
                    for j, (s, w) in enumerate(cks):
                        nc.tensor.matmul(out=pa[j][:], lhsT=xt[:],
                                         rhs=Wsb[:, kc, s:s + w],
                                         start=(kc == 0), stop=(kc == K - 1))
                    nc.tensor.matmul(out=pe[:], lhsT=xt[:], rhs=Wrsb[:, kc, :],
                                     start=(kc == 0), stop=(kc == K - 1))
                ft = op.tile([128, ROW], bf, tag="ft")
                for j, (s, w) in enumerate(cks):
                    nc.vector.tensor_copy(ft[:, s:s + w], pa[j][:])
                ert = op.tile([128, 4], bf, tag="ert")
                nc.vector.tensor_copy(ert[:], pe[:])
                nc.sync.dma_start(feat[t * 128:(t + 1) * 128, :ROW], ft[:])
                nc.sync.dma_start(er[t * 128:(t + 1) * 128, :], ert[:])
    nc.compile()
    return nc


def build_L(cfg, Dout):
    import concourse.bass as bass
    import concourse.tile as tile
    from concourse import bacc, mybir

    bf = mybir.dt.bfloat16
    f32 = mybir.dt.float32
    ROW = _row_elems(Dout)
    O = Dout // H
    nc = bacc.Bacc("TRN2", target_bir_lowering=False, debug=False,
                   num_devices=cfg.ncores)
    ftab = nc.dram_tensor("ftab", [cfg.nstar, ROW], bf, kind="ExternalInput").ap()
    ero = nc.dram_tensor("ero", [cfg.nloc, 4], bf, kind="ExternalInput").ap()
    idx = nc.dram_tensor("idx", [128, cfg.nch], mybir.dt.int32,
                         kind="ExternalInput").ap()
    OH = nc.dram_tensor("OH", [cfg.nch, 128, 128], bf, kind="ExternalInput").ap()
    OHT = nc.dram_tensor("OHT", [cfg.nch, 128, 128], bf, kind="ExternalInput").ap()
    brow = nc.dram_tensor("brow", [1, Dout], bf, kind="ExternalInput").ap()
    ones1 = nc.dram_tensor("ones1", [1, 128], bf, kind="ExternalInput").ap()
    xn = nc.dram_tensor("xn", [cfg.nloc, Dout], bf, kind="ExternalOutput").ap()
    rcks = _colchunks(Dout)
    with tile.TileContext(nc) as tc:
        with tc.tile_pool(name="c", bufs=1) as cp, \
             tc.tile_pool(name="g", bufs=2 * cfg.cpt + 2) as gp, \
             tc.tile_pool(name="oh", bufs=4) as ohp, \
             tc.tile_pool(name="s", bufs=2) as sp, \
             tc.tile_pool(name="ps", bufs=2, space="PSUM") as pp:
            idxsb = cp.tile([128, cfg.nch], mybir.dt.int32)
            nc.sync.dma_start(idxsb[:], idx[:])
            ersb = cp.tile([128, cfg.ntiles * 4], bf)
            for t in range(cfg.ntiles):
                nc.sync.dma_start(ersb[:, t * 4:(t + 1) * 4],
                                  ero[t * 128:(t + 1) * 128, :])
            on1 = cp.tile([1, 128], bf)
            nc.sync.dma_start(on1[:], ones1[:])
            brsb = cp.tile([1, Dout], bf)
            nc.sync.dma_start(brsb[:], brow[:])
            bps = pp.tile([128, Dout], f32, tag="bias", bufs=1)
            for (s, w) in rcks:
                nc.tensor.matmul(out=bps[:, s:s + w], lhsT=on1[:],
                                 rhs=brsb[:, s:s + w], start=True, stop=True)
            bsb = cp.tile([128, Dout], f32)
            nc.vector.tensor_copy(bsb[:], bps[:])
            for t in range(cfg.ntiles):
                gts = []
                erps = pp.tile([128, 64], f32, tag="erps")
                for c in range(cfg.cpt):
                    ch = t * cfg.cpt + c
                    gt = gp.tile([128, ROW], bf, tag="g")
                    nc.gpsimd.indirect_dma_start(
                        out=gt[:], out_offset=None, in_=ftab[:],
                        in_offset=bass.IndirectOffsetOnAxis(
                            ap=idxsb[:, ch:ch + 1], axis=0))
                    gts.append(gt)
                    oht = ohp.tile([128, 128], bf, tag="oht")
                    nc.sync.dma_start(oht[:], OHT[ch])
                    nc.tensor.matmul(out=erps[:, c * 4:(c + 1) * 4], lhsT=oht[:],
                                     rhs=ersb[:, t * 4:(t + 1) * 4],
                                     start=True, stop=True)
                zz = sp.tile([128, 64], f32, tag="zz")
                for c in range(cfg.cpt):
                    nc.vector.tensor_add(zz[:, c * 4:(c + 1) * 4],
                                         gts[c][:, Dout:Dout + 4],
                                         erps[:, c * 4:(c + 1) * 4])
                za = sp.tile([128, 64], f32, tag="za")
                nc.vector.scalar_tensor_tensor(
                    out=za[:], in0=zz[:], scalar=NEG_SLOPE, in1=zz[:],
                    op0=mybir.AluOpType.mult, op1=mybir.AluOpType.max)
                ee = sp.tile([128, 64], bf, tag="ee")
                nc.scalar.activation(ee[:], za[:],
                                     mybir.ActivationFunctionType.Exp)
                denps = pp.tile([128, 4], f32, tag="den")
                rstps = [pp.tile([128, w], f32, tag=f"rst{j}", name=f"rst{j}", bufs=1)
                         for j, (s, w) in enumerate(rcks)]
                for c in range(cfg.cpt):
                    gt = gts[c]
                    for h in range(H):
                        nc.vector.scalar_tensor_tensor(
                            out=gt[:, h * O:(h + 1) * O],
                            in0=gt[:, h * O:(h + 1) * O], scalar=1.0,
                            in1=ee[:, c * 4 + h:c * 4 + h + 1].to_broadcast(
                                [128, O]),
                            op0=mybir.AluOpType.mult, op1=mybir.AluOpType.mult)
                    ohc = ohp.tile([128, 128], bf, tag="ohc")
                    nc.sync.dma_start(ohc[:], OH[t * cfg.cpt + c])
                    nc.tensor.matmul(out=denps[:], lhsT=ohc[:],
                                     rhs=ee[:, c * 4:(c + 1) * 4],
                                     start=(c == 0), stop=(c == cfg.cpt - 1))
                    for j, (s, w) in enumerate(rcks):
                        nc.tensor.matmul(out=rstps[j][:], lhsT=ohc[:],
                                         rhs=gt[:, s:s + w],
                                         start=(c == 0), stop=(c == cfg.cpt - 1))
                dcl = sp.tile([128, 4], f32, tag="dcl")
                nc.vector.tensor_scalar_max(dcl[:], denps[:], 1e-9)
                rec = sp.tile([128, 4], f32, tag="rec")
                nc.vector.reciprocal(rec[:], dcl[:])
                y = sp.tile([128, Dout], f32, tag="y")
                for h in range(H):
                    j = (h * O) // 512
                    s0 = (h * O) % 512
                    nc.vector.scalar_tensor_tensor(
                        out=y[:, h * O:(h + 1) * O], in0=rstps[j][:, s0:s0 + O],
                        scalar=rec[:, h:h + 1], in1=bsb[:, h * O:(h + 1) * O],
                        op0=mybir.AluOpType.mult, op1=mybir.AluOpType.add)
                mn = sp.tile([128, Dout], f32, tag="mn")
                nc.vector.tensor_scalar_min(mn[:], y[:], 0.0)
                ex = sp.tile([128, Dout], f32, tag="ex")
                nc.scalar.activation(ex[:], mn[:],
                                     mybir.ActivationFunctionType.Exp)
                y2 = sp.tile([128, Dout], f32, tag="y2")
                nc.vector.scalar_tensor_tensor(
                    out=y2[:], in0=y[:], scalar=0.0, in1=ex[:],
                    op0=mybir.AluOpType.max, op1=mybir.AluOpType.add)
                xo = sp.tile([128, Dout], bf, tag="xo")
                nc.vector.tensor_scalar_add(xo[:], y2[:], -1.0)
                nc.sync.dma_start(xn[t * 128:(t + 1) * 128, :], xo[:])
    nc.compile()
    return nc


def build_POOL(cfg):
    import concourse.tile as tile
    from concourse import bacc, mybir

    bf = mybir.dt.bfloat16
    f32 = mybir.dt.float32
    nc = bacc.Bacc("TRN2", target_bir_lowering=False, debug=False,
                   num_devices=cfg.ncores)
    h3T = nc.dram_tensor("h3T", [128, 8 * cfg.nloc], bf, kind="ExternalInput").ap()
    h3 = nc.dram_tensor("h3", [cfg.nloc, D3], bf, kind="ExternalInput").ap()
    Wg1 = nc.dram_tensor("Wg1", [128, 8, 128], bf, kind="ExternalInput").ap()
    bg1c = nc.dram_tensor("bg1c", [128, 1], f32, kind="ExternalInput").ap()
    Wg2c = nc.dram_tensor("Wg2c", [128, 1], bf, kind="ExternalInput").ap()
    bg2r = nc.dram_tensor("bg2r", [128, 1], f32, kind="ExternalInput").ap()
    GOH = nc.dram_tensor("GOH", [cfg.ntiles, 128, cfg.gpad], bf,
                         kind="ExternalInput").ap()
    Wmu = nc.dram_tensor("Wmu", [128, 8, 128], f32, kind="ExternalInput").ap()
    Wlv = nc.dram_tensor("Wlv", [128, 8, 128], f32, kind="ExternalInput").ap()
    bmu = nc.dram_tensor("bmu", [1, 128], f32, kind="ExternalInput").ap()
    blv = nc.dram_tensor("blv", [1, 128], f32, kind="ExternalInput").ap()
    on32 = nc.dram_tensor("on32", [1, 32], f32, kind="ExternalInput").ap()
    identd = nc.dram_tensor("identd", [32, 32], f32, kind="ExternalInput").ap()
    mu = nc.dram_tensor("mu", [cfg.gpad, 128], f32, kind="ExternalOutput").ap()
    lv = nc.dram_tensor("lv", [cfg.gpad, 128], f32, kind="ExternalOutput").ap()
    nwin = (cfg.nloc + 511) // 512
    with tile.TileContext(nc) as tc:
        with tc.tile_pool(name="c", bufs=1) as cp, \
             tc.tile_pool(name="s", bufs=3) as sp, \
             tc.tile_pool(name="ps", bufs=1, space="PSUM") as pp:
            Wg1s = cp.tile([128, 8, 128], bf)
            nc.sync.dma_start(Wg1s[:], Wg1[:])
            h3Ts = cp.tile([128, 8 * cfg.nloc], bf)
            nc.sync.dma_start(h3Ts[:], h3T[:])
            small = {}
            for nm, ap_, dt_ in [("bg1c", bg1c, f32), ("Wg2c", Wg2c, bf),
                                 ("bg2r", bg2r, f32), ("on32", on32, f32),
                                 ("bmu", bmu, f32), ("blv", blv, f32)]:
                tl = cp.tile(list(ap_.shape), dt_, tag=nm, name=nm)
                nc.sync.dma_start(tl[:], ap_[:])
                small[nm] = tl
            GOHs = cp.tile([128, cfg.ntiles * cfg.gpad], bf)
            for t in range(cfg.ntiles):
                nc.sync.dma_start(GOHs[:, t * cfg.gpad:(t + 1) * cfg.gpad],
                                  GOH[t])
            relu1 = cp.tile([128, cfg.nloc], bf)
            for w in range(nwin):
                s = w * 512
                ww = min(512, cfg.nloc - s)
                ps = pp.tile([128, 512], f32, tag="g1")
                for kc in range(8):
                    nc.tensor.matmul(out=ps[:, :ww], lhsT=Wg1s[:, kc, :],
                                     rhs=h3Ts[:, kc * cfg.nloc + s:
                                              kc * cfg.nloc + s + ww],
                                     start=(kc == 0), stop=(kc == 7))
                nc.scalar.activation(relu1[:, s:s + ww], ps[:, :ww],
                                     mybir.ActivationFunctionType.Relu,
                                     bias=small["bg1c"][:])
            gps = pp.tile([128, 32], f32, tag="g2")
            for t in range(cfg.ntiles):
                nc.tensor.matmul(out=gps[:, t:t + 1],
                                 lhsT=relu1[:, t * 128:(t + 1) * 128],
                                 rhs=small["Wg2c"][:], start=True, stop=True)
            eg = sp.tile([128, cfg.ntiles], bf, tag="eg")
            nc.scalar.activation(eg[:], gps[:, :cfg.ntiles],
                                 mybir.ActivationFunctionType.Exp,
                                 bias=small["bg2r"][:])
            gd = pp.tile([cfg.gpad, 1], f32, tag="gd")
            goha = sp.tile([128, cfg.ntiles * cfg.gpad], bf, tag="goha")
            for t in range(cfg.ntiles):
                nc.tensor.matmul(out=gd[:], lhsT=GOHs[:, t * cfg.gpad:
                                                      (t + 1) * cfg.gpad],
                                 rhs=eg[:, t:t + 1],
                                 start=(t == 0), stop=(t == cfg.ntiles - 1))
                nc.vector.tensor_mul(
                    goha[:, t * cfg.gpad:(t + 1) * cfg.gpad],
                    GOHs[:, t * cfg.gpad:(t + 1) * cfg.gpad],
                    eg[:, t:t + 1].to_broadcast([128, cfg.gpad]))
            h3s = sp.tile([128, D3], bf, tag="h3s")
            geps = [pp.tile([cfg.gpad, 512], f32, tag=f"ge{j}", name=f"geps{j}") for j in range(2)]
            for t in range(cfg.ntiles):
                h3t = sp.tile([128, D3], bf, tag="h3t")
                nc.sync.dma_start(h3t[:], h3[t * 128:(t + 1) * 128, :])
                for j in range(2):
                    nc.tensor.matmul(out=geps[j][:],
                                     lhsT=goha[:, t * cfg.gpad:(t + 1) * cfg.gpad],
                                     rhs=h3t[:, j * 512:(j + 1) * 512],
                                     start=(t == 0), stop=(t == cfg.ntiles - 1))
            gdc = sp.tile([cfg.gpad, 1], f32, tag="gdc")
            nc.vector.tensor_scalar_max(gdc[:], gd[:], 1e-9)
            grc = sp.tile([cfg.gpad, 1], f32, tag="grc")
            nc.vector.reciprocal(grc[:], gdc[:])
            zge = sp.tile([cfg.gpad, D3], f32, tag="zge")
            nc.vector.memset(zge[:], 0.0)
            ge = sp.tile([cfg.gpad, D3], f32, tag="ge")
            for j in range(2):
                nc.vector.scalar_tensor_tensor(
                    out=ge[:, j * 512:(j + 1) * 512], in0=geps[j][:],
                    scalar=grc[:, 0:1], in1=zge[:, j * 512:(j + 1) * 512],
                    op0=mybir.AluOpType.mult, op1=mybir.AluOpType.add)
            # transpose ge via PE (fp32): [gpad,128]-chunks -> geT [128, 8*gpad]
            if True:
                ident = cp.tile([cfg.gpad, cfg.gpad], f32, tag="ident")
                nc.sync.dma_start(ident[:], identd[:])
                geT = sp.tile([128, 8 * cfg.gpad], f32, tag="geT")
                for kc in range(8):
                    pst = pp.tile([128, cfg.gpad], f32, tag="pst")
                    nc.tensor.transpose(out=pst[:],
                                        in_=ge[:, kc * 128:(kc + 1) * 128],
                                        identity=ident[:])
                    nc.vector.tensor_copy(geT[:, kc * cfg.gpad:(kc + 1) * cfg.gpad],
                                          pst[:])
                for nm, Wt, bt, outp in [("mu", Wmu, "bmu", mu),
                                         ("lv", Wlv, "blv", lv)]:
                    Ws = sp.tile([128, 8, 128], f32, tag="Wmlv")
                    nc.sync.dma_start(Ws[:], Wt[:])
                    mps = pp.tile([cfg.gpad, 128], f32, tag="mps")
                    for kc in range(8):
                        nc.tensor.matmul(out=mps[:],
                                         lhsT=geT[:, kc * cfg.gpad:(kc + 1) * cfg.gpad],
                                         rhs=Ws[:, kc, :],
                                         start=(kc == 0), stop=False)
                    nc.tensor.matmul(out=mps[:], lhsT=small["on32"][:],
                                     rhs=small[bt][:],
                                     start=False, stop=True)
                    mo = sp.tile([cfg.gpad, 128], f32, tag="mo")
                    nc.vector.tensor_copy(mo[:], mps[:])
                    nc.sync.dma_start(outp[:], mo[:])
    nc.compile()
    return nc


_BUILD_CACHE = {}


def _get(key, fn):
    if key not in _BUILD_CACHE:
        _BUILD_CACHE[key] = fn()
    return _BUILD_CACHE[key]


def _run(nc, in_maps):
    from concourse.bass_utils import run_bass_kernel_spmd
    return run_bass_kernel_spmd(nc, in_maps, core_ids=list(range(NCORES))).results


# ------------------------------------------------------------------ main entry
def kernel(node_feat, src, dst, graph_ids,
           W1, al1, ar1, b1, W2, al2, ar2, b2, W3, al3, ar3, b3,
           Wg1, bg1, Wg2, bg2, Wmu, bmu, Wlv, blv, cfg=None):
    cfg = cfg or CFG_FULL
    nc_ = cfg.ncores
    prep = host_prep(cfg, node_feat, src, dst, graph_ids)
    layers = [(np.asarray(W1, np.float32), np.asarray(al1, np.float32),
               np.asarray(ar1, np.float32), np.asarray(b1, np.float32)),
              (np.asarray(W2, np.float32), np.asarray(al2, np.float32),
               np.asarray(ar2, np.float32), np.asarray(b2, np.float32)),
              (np.asarray(W3, np.float32), np.asarray(al3, np.float32),
               np.asarray(ar3, np.float32), np.asarray(b3, np.float32))]
    douts = [D1, D2, D3]

    xblocks = [np.ascontiguousarray(prep["x1"][c * cfg.nloc:(c + 1) * cfg.nloc])
               for c in range(nc_)]
    for li, (W, al, ar, b) in enumerate(layers):
        Din, Dout = W.shape
        ROW = _row_elems(Dout)
        Waug, Wr = fold_weights(W, al, ar)
        ncP = _get(("P", Din, Dout), lambda: build_P(cfg, Din, Dout))
        inP = [dict(xT=xT_own_blocks(cfg, xblocks[c]).astype(BF16),
                    Wa=wstack(Waug).astype(BF16), Wr=wstack(Wr).astype(BF16))
               for c in range(nc_)]
        outP = _run(ncP, inP)
        ftab = np.concatenate([outP[c]["feat"] for c in range(nc_)], 0)
        ncL = _get(("L", Dout), lambda: build_L(cfg, Dout))
        inL = [dict(ftab=ftab, ero=outP[c]["er"], idx=prep["idx32"][c],
                    OH=prep["oh"][c], OHT=prep["oht"][c],
                    brow=b[None].astype(BF16),
                    ones1=np.ones((1, 128), BF16))
               for c in range(nc_)]
        outL = _run(ncL, inL)
        xblocks = [outL[c]["xn"].astype(np.float32) for c in range(nc_)]

    ncPool = _get(("POOL",), lambda: build_POOL(cfg))
    Wg1f = np.asarray(Wg1, np.float32)
    inPool = [dict(
        h3T=xT_own_blocks(cfg, xblocks[c]).astype(BF16),
        h3=xblocks[c].astype(BF16),
        Wg1=wstack(Wg1f).astype(BF16),
        bg1c=np.asarray(bg1, np.float32).reshape(128, 1),
        Wg2c=np.asarray(Wg2, BF16).reshape(128, 1),
        bg2r=np.full((128, 1), np.asarray(bg2, np.float32).reshape(-1)[0],
                     np.float32),
        GOH=prep["goh"][c],
        Wmu=wstack(np.asarray(Wmu, np.float32)),
        Wlv=wstack(np.asarray(Wlv, np.float32)),
        bmu=np.asarray(bmu, np.float32)[None],
        blv=np.asarray(blv, np.float32)[None],
        on32=np.ones((1, 32), np.float32),
        identd=np.eye(32, dtype=np.float32)) for c in range(nc_)]
    outPool = _run(ncPool, inPool)
    mu = np.concatenate([outPool[c]["mu"][:cfg.gpc] for c in range(nc_)], 0)
    lv = np.concatenate([outPool[c]["lv"][:cfg.gpc] for c in range(nc_)], 0)
    return np.asarray(mu, np.float32), np.asarray(lv, np.float32)



# revision 18
# speedup vs baseline: 36.3473x; 36.3473x over previous
# kernel.py — CrystalGCNEncoder (3-layer GAT + global attention pooling) on 8 trn2
# NeuronCores, fully fused into ONE SPMD launch.
#
# The previous version ran 7 separate launches and shipped the replicated
# per-layer feature table (plus dense one-hot scatter matrices) from the host
# every layer — hundreds of MB over the axon tunnel per call.  This version
# keeps everything on device:
#   - each core owns 25 graphs' nodes (slots balanced by in-degree over 21
#     tiles of 128) and all edges whose dst lives there
#   - per layer: local fc matmuls (P), AllGather of the [nstar, ROW] feature
#     table into Shared DRAM, then edge gather + softmax + one-hot-matmul
#     aggregation (L) — one-hot matrices are built on device from int32
#     dst-position indices via iota + is_equal
#   - weights ship sharded (1/8th per core) and are AllGathered on device
#   - pooling/readout fully local; host only concatenates the [32,128] outputs
# H2D is ~11 MB total instead of ~1.5 GB.
import numpy as np
import ml_dtypes

N, E, G = 20000, 320000, 200
F_IN, HID, H, LAT = 128, 128, 4, 128
O1, O2, O3 = HID // 2, HID, 2 * HID
D1, D2, D3 = H * O1, H * O2, H * O3          # 256, 512, 1024
NEG_SLOPE = 0.2
NCORES = 8
BF16 = ml_dtypes.bfloat16

NTILES = 21
NLOC = NTILES * 128          # 2688 slots per core
NSTAR = NLOC * NCORES        # 21504
CPT = 16                     # edge chunks (of 128) per tile
TPE = CPT * 128              # 2048 edges per tile
ELOC = NTILES * TPE          # 43008 edge slots per core
NCH = ELOC // 128            # 336 chunks per core
GPC = G // NCORES            # 25 graphs per core
GPAD = 32

LAYERS = [(F_IN, D1), (D1, D2), (D2, D3)]    # (Din, Dout)


def _row_elems(d):          # feat row: [d feats | 4 el | 4 ee | pad] bf16
    b = (d + 8) * 2
    return ((b + 255) // 256 * 256) // 2


ROWS = [_row_elems(d) for _, d in LAYERS]    # 384, 640, 1152


def _colchunks(c):
    out, s = [], 0
    while s < c:
        w = min(512, c - s)
        out.append((s, w))
        s += w
    return out


# ------------------------------------------------------------- weight layout
def _wlayout():
    """(bf16 offsets, bf16 total, f32 offsets, f32 total) of the packed,
    core-sharded weight buffers."""
    offb, ob = {}, 0
    for nm, n in [("Wa1", 128 * 1 * (D1 + 8)), ("Wa2", 128 * 2 * (D2 + 8)),
                  ("Wa3", 128 * 4 * (D3 + 8)), ("b1", D1), ("b2", D2),
                  ("b3", D3), ("Wg1", 128 * 8 * 128), ("Wg2", 128)]:
        offb[nm] = ob
        ob += n
    offf, of = {}, 0
    for nm, n in [("Wmu", 128 * 8 * 128), ("Wlv", 128 * 8 * 128),
                  ("bg1", 128), ("bg2", 128), ("bmu", 128), ("blv", 128)]:
        offf[nm] = of
        of += n
    pb = -(-ob // NCORES) * NCORES
    pf = -(-of // NCORES) * NCORES
    return offb, pb, offf, pf


OFFB, TOTB, OFFF, TOTF = _wlayout()
PB, PF = TOTB // NCORES, TOTF // NCORES


# ------------------------------------------------------------------ host prep
def host_prep(node_feat, src, dst, graph_ids):
    node_feat = np.asarray(node_feat, np.float32)
    src = np.asarray(src).astype(np.int64)
    dst = np.asarray(dst).astype(np.int64)
    graph_ids = np.asarray(graph_ids).astype(np.int64)

    gbounds = np.arange(NCORES + 1) * GPC
    nbounds = np.searchsorted(graph_ids, gbounds)
    core_of_node = np.searchsorted(nbounds, np.arange(N), side="right") - 1
    indeg = np.bincount(dst, minlength=N)

    glob2slot = np.zeros(N, np.int64)
    tile_of_node = np.zeros(N, np.int64)
    slotpos_of_node = np.zeros(N, np.int64)
    for c in range(NCORES):
        nodes = np.arange(nbounds[c], nbounds[c + 1])
        assert len(nodes) <= NLOC
        order = nodes[np.argsort(-indeg[nodes], kind="stable")]
        loads = np.zeros(NTILES, np.int64)
        counts = np.zeros(NTILES, np.int64)
        for nd in order:
            free = np.nonzero(counts < 128)[0]
            tgt = free[np.argmin(loads[free])]
            tile_of_node[nd] = tgt
            slotpos_of_node[nd] = counts[tgt]
            glob2slot[nd] = c * NLOC + tgt * 128 + counts[tgt]
            counts[tgt] += 1
            loads[tgt] += indeg[nd]
        assert loads.max() <= TPE

    edge_core = core_of_node[dst]
    idx_l, dp_l, gid_l = [], [], []
    for c in range(NCORES):
        eids = np.nonzero(edge_core == c)[0]
        assert len(eids) <= ELOC
        src_slot = np.zeros(ELOC, np.int64)
        dst_pos = np.full(ELOC, -1, np.int64)
        et = tile_of_node[dst[eids]]
        for t in range(NTILES):
            sel = eids[et == t]
            assert len(sel) <= TPE
            b = t * TPE
            src_slot[b : b + len(sel)] = glob2slot[src[sel]]
            dst_pos[b : b + len(sel)] = slotpos_of_node[dst[sel]]
        idx_l.append(np.ascontiguousarray(
            src_slot.reshape(NCH, 128).T).astype(np.int32))
        dp_l.append(np.ascontiguousarray(
            dst_pos.reshape(NCH, 128).T).astype(np.int32))
        gid = np.full((128, NTILES), -1, np.int64)
        nodes = np.arange(nbounds[c], nbounds[c + 1])
        gid[slotpos_of_node[nodes], tile_of_node[nodes]] = \
            graph_ids[nodes] - c * GPC
        gid_l.append(gid.astype(np.int32))

    x1 = np.zeros((NSTAR, F_IN), np.float32)
    x1[glob2slot] = node_feat
    x1T_l = [np.ascontiguousarray(x1[c * NLOC:(c + 1) * NLOC].T).astype(BF16)
             for c in range(NCORES)]
    return dict(idx=idx_l, dp=dp_l, gid=gid_l, x1T=x1T_l)


def fold_weights(W, al, ar):
    """[Din, Dout] (+attn vecs) -> [Din, Dout+8] = [W | W@al_h | W@ar_h]."""
    Din, D = W.shape
    Hh, O = al.shape
    Wl = np.einsum("iho,ho->ih", W.reshape(Din, Hh, O), al)
    Wr = np.einsum("iho,ho->ih", W.reshape(Din, Hh, O), ar)
    return np.concatenate([W, Wl, Wr], 1)


def wstack_flat(Waug):
    """[Din, C] -> flat (p, k, c)-major [128 * Din/128 * C]."""
    Din, C = Waug.shape
    return np.ascontiguousarray(
        Waug.reshape(Din // 128, 128, C).transpose(1, 0, 2)).reshape(-1)


def pack_weights(Ws, als, ars, bs, Wg1, bg1, Wg2, bg2, Wmu, bmu, Wlv, blv):
    bfb = np.zeros(TOTB, BF16)
    def putb(nm, a):
        a = np.asarray(a, np.float32).reshape(-1)
        bfb[OFFB[nm]:OFFB[nm] + len(a)] = a.astype(BF16)
    for i in range(3):
        putb(f"Wa{i+1}", wstack_flat(fold_weights(Ws[i], als[i], ars[i])))
        putb(f"b{i+1}", bs[i])
    putb("Wg1", wstack_flat(np.asarray(Wg1, np.float32)))
    putb("Wg2", Wg2)
    f32b = np.zeros(TOTF, np.float32)
    def putf(nm, a):
        a = np.asarray(a, np.float32).reshape(-1)
        f32b[OFFF[nm]:OFFF[nm] + len(a)] = a
    putf("Wmu", wstack_flat(np.asarray(Wmu, np.float32)))
    putf("Wlv", wstack_flat(np.asarray(Wlv, np.float32)))
    putf("bg1", bg1)
    putf("bg2", np.full(128, np.asarray(bg2, np.float32).reshape(-1)[0]))
    putf("bmu", bmu)
    putf("blv", blv)
    return bfb.reshape(NCORES, PB), f32b.reshape(NCORES, PF)


# ------------------------------------------------------------------ builder
def build_fused():
    import concourse.bass as bass
    import concourse.tile as tile
    from concourse import bacc, mybir

    bf = mybir.dt.bfloat16
    f32 = mybir.dt.float32
    i32 = mybir.dt.int32
    AF = mybir.ActivationFunctionType
    ALU = mybir.AluOpType
    RG = [list(range(NCORES))]

    nc = bacc.Bacc("TRN2", target_bir_lowering=False, debug=False,
                   num_devices=NCORES)
    x1T = nc.dram_tensor("x1T", [128, NLOC], bf, kind="ExternalInput").ap()
    idx = nc.dram_tensor("idx", [128, NCH], i32, kind="ExternalInput").ap()
    dp = nc.dram_tensor("dp", [128, NCH], i32, kind="ExternalInput").ap()
    gid = nc.dram_tensor("gid", [128, NTILES], i32, kind="ExternalInput").ap()
    wbf = nc.dram_tensor("wbf", [PB], bf, kind="ExternalInput").ap()
    wf32 = nc.dram_tensor("wf32", [PF], f32, kind="ExternalInput").ap()
    mu = nc.dram_tensor("mu", [GPAD, 128], f32, kind="ExternalOutput").ap()
    lv = nc.dram_tensor("lv", [GPAD, 128], f32, kind="ExternalOutput").ap()

    wbl = nc.dram_tensor("wbl", [PB], bf, kind="Internal").ap()
    wbs = nc.dram_tensor("wbs", [TOTB], bf, kind="Internal",
                         addr_space="Shared").ap()
    wfl = nc.dram_tensor("wfl", [PF], f32, kind="Internal").ap()
    wfs = nc.dram_tensor("wfs", [TOTF], f32, kind="Internal",
                         addr_space="Shared").ap()
    flocs, ftabs = [], []
    for l in range(3):
        flocs.append(nc.dram_tensor(f"floc{l}", [NLOC, ROWS[l]], bf,
                                    kind="Internal").ap())
        ftabs.append(nc.dram_tensor(f"ftab{l}", [NSTAR, ROWS[l]], bf,
                                    kind="Internal", addr_space="Shared").ap())

    with tile.TileContext(nc) as tc:
        with tc.tile_pool(name="cst", bufs=1) as cp, \
             tc.tile_pool(name="xper", bufs=1) as xp, \
             tc.tile_pool(name="trp", bufs=2, space="PSUM") as trp:
            # ---- stage + allgather weights
            nc.sync.dma_start(wbl[:], wbf[:])
            nc.sync.dma_start(wfl[:], wf32[:])
            nc.gpsimd.collective_compute(
                "AllGather", ALU.bypass, replica_groups=RG,
                ins=[wbl[:]], outs=[wbs[:]])
            nc.gpsimd.collective_compute(
                "AllGather", ALU.bypass, replica_groups=RG,
                ins=[wfl[:]], outs=[wfs[:]])

            def segb(nm, shape):
                n = int(np.prod(shape))
                a = wbs[OFFB[nm]:OFFB[nm] + n]
                if len(shape) == 3:
                    return a.rearrange("(p k c) -> p k c", p=shape[0],
                                       k=shape[1], c=shape[2])
                return a.rearrange("(a c) -> a c", a=shape[0], c=shape[1])

            def segf(nm, shape):
                n = int(np.prod(shape))
                a = wfs[OFFF[nm]:OFFF[nm] + n]
                if len(shape) == 3:
                    return a.rearrange("(p k c) -> p k c", p=shape[0],
                                       k=shape[1], c=shape[2])
                return a.rearrange("(a c) -> a c", a=shape[0], c=shape[1])

            # ---- constants
            iota_row = cp.tile([128, 128], i32, tag="io_r")
            nc.gpsimd.iota(iota_row[:], pattern=[[1, 128]], base=0,
                           channel_multiplier=0)
            iota_col = cp.tile([128, 128], i32, tag="io_c")
            nc.gpsimd.iota(iota_col[:], pattern=[[0, 128]], base=0,
                           channel_multiplier=1)
            iota16 = iota_row[:].unsqueeze(1).broadcast_to([128, CPT, 128])
            ident = cp.tile([128, 128], bf, tag="ident")
            nc.vector.tensor_tensor(out=ident[:], in0=iota_row[:],
                                    in1=iota_col[:], op=ALU.is_equal)
            ident32 = cp.tile([32, 32], f32, tag="id32")
            nc.vector.tensor_tensor(out=ident32[:], in0=iota_row[0:32, 0:32],
                                    in1=iota_col[0:32, 0:32], op=ALU.is_equal)
            on1 = cp.tile([1, 128], bf, tag="on1")
            nc.vector.memset(on1[:], 1.0)
            on32 = cp.tile([1, 32], f32, tag="on32")
            nc.vector.memset(on32[:], 1.0)

            idxsb = cp.tile([128, NCH], i32, tag="idx")
            nc.sync.dma_start(idxsb[:], idx[:])
            dpsb = cp.tile([128, NCH], i32, tag="dp")
            nc.sync.dma_start(dpsb[:], dp[:])
            gidsb = cp.tile([128, NTILES], i32, tag="gid")
            nc.sync.dma_start(gidsb[:], gid[:])

            xtiles = {1: [], 2: [], 3: []}   # layer outputs [128, Dout]/tile

            # ================================================== layers
            for l, (Din, Dout) in enumerate(LAYERS):
                K, O, ROW = Din // 128, Dout // H, ROWS[l]
                fcks = _colchunks(Dout + 8)
                with tc.tile_pool(name=f"l{l}c", bufs=1) as lc:
                    ers = lc.tile([128, NTILES * 4], bf, tag="ers")
                    bsb = lc.tile([128, Dout], bf, tag="bsb")
                    Wa = lc.tile([128, K, Dout + 8], bf, tag="Wa")
                    nc.sync.dma_start(Wa[:], segb(f"Wa{l+1}",
                                                  [128, K, Dout + 8]))
                    brow = lc.tile([1, Dout], bf, tag="br")
                    nc.sync.dma_start(brow[:], segb(f"b{l+1}", [1, Dout]))
                    if l == 0:
                        x1Ts = lc.tile([128, NLOC], bf, tag="x1T")
                        nc.sync.dma_start(x1Ts[:], x1T[:])
                    # ---------- P phase: feat/el/er matmuls on own nodes
                    with tc.tile_pool(name=f"P{l}s", bufs=3) as sp, \
                         tc.tile_pool(name=f"P{l}p", bufs=1,
                                      space="PSUM") as pp:
                        bps = pp.tile([128, Dout], f32, tag="bps")
                        for (s, w) in _colchunks(Dout):
                            nc.tensor.matmul(out=bps[:, s:s + w],
                                             lhsT=on1[:],
                                             rhs=brow[:, s:s + w],
                                             start=True, stop=True)
                        nc.vector.tensor_copy(bsb[:], bps[:])
                        for t in range(NTILES):
                            pas = [pp.tile([128, w], f32, tag=f"pa{j}",
                                           name=f"pa{j}")
                                   for j, (s, w) in enumerate(fcks)]
                            for kc in range(K):
                                if l == 0:
                                    xt = x1Ts[:, t * 128:(t + 1) * 128]
                                else:
                                    ptr = trp.tile([128, 128], bf, tag="tr")
                                    nc.tensor.transpose(
                                        out=ptr[:],
                                        in_=xtiles[l][t][:, kc * 128:
                                                         (kc + 1) * 128],
                                        identity=ident[:])
                                    xts = sp.tile([128, 128], bf, tag="xt")
                                    nc.scalar.activation(xts[:], ptr[:],
                                                         AF.Copy)
                                    xt = xts[:]
                                for j, (s, w) in enumerate(fcks):
                                    nc.tensor.matmul(
                                        out=pas[j][:], lhsT=xt,
                                        rhs=Wa[:, kc, s:s + w],
                                        start=(kc == 0), stop=(kc == K - 1))
                            ft = sp.tile([128, ROW], bf, tag="ft")
                            for j, (s, w) in enumerate(fcks):
                                fw = min(s + w, Dout + 4) - s
                                if fw > 0:
                                    nc.vector.tensor_copy(ft[:, s:s + fw],
                                                          pas[j][:, 0:fw])
                            nc.vector.tensor_copy(
                                ers[:, t * 4:(t + 1) * 4],
                                pas[-1][:, fcks[-1][1] - 4:fcks[-1][1]])
                            nc.sync.dma_start(
                                flocs[l][t * 128:(t + 1) * 128, :], ft[:])
                    # ---------- AllGather feature table
                    nc.gpsimd.collective_compute(
                        "AllGather", ALU.bypass, replica_groups=RG,
                        ins=[flocs[l][:]], outs=[ftabs[l][:]])
                    # ---------- L phase: gather, edge softmax, aggregate
                    with tc.tile_pool(name=f"L{l}g", bufs=2) as gp, \
                         tc.tile_pool(name=f"L{l}o", bufs=2) as ohp, \
                         tc.tile_pool(name=f"L{l}t", bufs=3) as otp, \
                         tc.tile_pool(name=f"L{l}s", bufs=2) as sp, \
                         tc.tile_pool(name=f"L{l}p", bufs=1,
                                      space="PSUM") as pp:
                        for t in range(NTILES):
                            gtt = gp.tile([128, CPT * ROW], bf, tag="g")
                            oh16 = ohp.tile([128, CPT * 128], bf, tag="oh")
                            nc.vector.tensor_tensor(
                                out=oh16[:].rearrange("p (c e) -> p c e",
                                                      c=CPT, e=128),
                                in0=dpsb[:, t * CPT:(t + 1) * CPT]
                                    .to_broadcast([128, CPT, 128]),
                                in1=iota16, op=ALU.is_equal)
                            erps = pp.tile([128, CPT * 4], f32, tag="erps")
                            for c in range(CPT):
                                ch = t * CPT + c
                                nc.gpsimd.indirect_dma_start(
                                    out=gtt[:, c * ROW:(c + 1) * ROW],
                                    out_offset=None, in_=ftabs[l][:],
                                    in_offset=bass.IndirectOffsetOnAxis(
                                        ap=idxsb[:, ch:ch + 1], axis=0))
                                ptr = trp.tile([128, 128], bf, tag="tr")
                                nc.tensor.transpose(
                                    out=ptr[:],
                                    in_=oh16[:, c * 128:(c + 1) * 128],
                                    identity=ident[:])
                                oht = otp.tile([128, 128], bf, tag="oht")
                                nc.scalar.activation(oht[:], ptr[:], AF.Copy)
                                nc.tensor.matmul(
                                    out=erps[:, c * 4:(c + 1) * 4],
                                    lhsT=oht[:],
                                    rhs=ers[:, t * 4:(t + 1) * 4],
                                    start=True, stop=True)
                            g3 = gtt[:].rearrange("p (c r) -> p c r",
                                                  c=CPT, r=ROW)
                            zz = sp.tile([128, CPT * 4], f32, tag="zz")
                            nc.vector.tensor_add(
                                zz[:].rearrange("p (c h) -> p c h",
                                                c=CPT, h=4),
                                g3[:, :, Dout:Dout + 4],
                                erps[:].rearrange("p (c h) -> p c h",
                                                  c=CPT, h=4))
                            za = sp.tile([128, CPT * 4], f32, tag="za")
                            nc.vector.scalar_tensor_tensor(
                                out=za[:], in0=zz[:], scalar=NEG_SLOPE,
                                in1=zz[:], op0=ALU.mult, op1=ALU.max)
                            ee = sp.tile([128, CPT * 4], bf, tag="ee")
                            nc.scalar.activation(ee[:], za[:], AF.Exp)
                            # ee into the row tail (feeds the den columns)
                            nc.vector.tensor_copy(
                                g3[:, :, Dout + 4:Dout + 8],
                                ee[:].rearrange("p (c h) -> p c h",
                                                c=CPT, h=4))
                            rstps = [pp.tile([128, w], f32, tag=f"rst{j}",
                                             name=f"rst{j}")
                                     for j, (s, w) in enumerate(fcks)]
                            for c in range(CPT):
                                e4 = ee[:, c * 4:(c + 1) * 4]
                                gslc = gtt[:, c * ROW:c * ROW + Dout] \
                                    .rearrange("p (h o) -> p h o", h=H, o=O)
                                nc.vector.scalar_tensor_tensor(
                                    out=gslc, in0=gslc, scalar=1.0,
                                    in1=e4.to_broadcast([128, H, O]),
                                    op0=ALU.mult, op1=ALU.mult)
                                for j, (s, w) in enumerate(fcks):
                                    nc.tensor.matmul(
                                        out=rstps[j][:],
                                        lhsT=oh16[:, c * 128:(c + 1) * 128],
                                        rhs=gtt[:, c * ROW + s:
                                                c * ROW + s + w],
                                        start=(c == 0), stop=(c == CPT - 1))
                            wl = fcks[-1][1]
                            dcl = sp.tile([128, 4], f32, tag="dcl")
                            nc.vector.tensor_scalar_max(
                                dcl[:], rstps[-1][:, wl - 4:wl], 1e-9)
                            rec = sp.tile([128, 4], f32, tag="rec")
                            nc.vector.reciprocal(rec[:], dcl[:])
                            y = sp.tile([128, Dout], f32, tag="y", bufs=1)
                            for h in range(H):
                                j = (h * O) // 512
                                s0 = (h * O) % 512
                                nc.vector.scalar_tensor_tensor(
                                    out=y[:, h * O:(h + 1) * O],
                                    in0=rstps[j][:, s0:s0 + O],
                                    scalar=rec[:, h:h + 1],
                                    in1=bsb[:, h * O:(h + 1) * O],
                                    op0=ALU.mult, op1=ALU.add)
                            mn = sp.tile([128, Dout], f32, tag="mn", bufs=1)
                            nc.vector.tensor_scalar_min(mn[:], y[:], 0.0)
                            nc.scalar.activation(mn[:], mn[:], AF.Exp)
                            nc.vector.scalar_tensor_tensor(
                                out=y[:], in0=y[:], scalar=0.0, in1=mn[:],
                                op0=ALU.max, op1=ALU.add)
                            xo = xp.tile([128, Dout], bf, tag=f"xo{l}_{t}",
                                         name=f"xo{l}_{t}")
                            nc.vector.tensor_scalar_add(xo[:], y[:], -1.0)
                            xtiles[l + 1].append(xo)

            # ================================================== pooling
            h3 = xtiles[3]
            with tc.tile_pool(name="pc", bufs=1) as pc, \
                 tc.tile_pool(name="ps", bufs=3) as sp:
                Wg1s = pc.tile([128, 8, 128], bf, tag="Wg1")
                nc.sync.dma_start(Wg1s[:], segb("Wg1", [128, 8, 128]))
                Wg2c = pc.tile([128, 1], bf, tag="Wg2")
                nc.sync.dma_start(Wg2c[:], segb("Wg2", [128, 1]))
                Wmus = pc.tile([128, 8, 128], f32, tag="Wmu")
                nc.sync.dma_start(Wmus[:], segf("Wmu", [128, 8, 128]))
                Wlvs = pc.tile([128, 8, 128], f32, tag="Wlv")
                nc.sync.dma_start(Wlvs[:], segf("Wlv", [128, 8, 128]))
                bg1c = pc.tile([128, 1], f32, tag="bg1")
                nc.sync.dma_start(bg1c[:], segf("bg1", [128, 1]))
                bg2r = pc.tile([128, 1], f32, tag="bg2")
                nc.sync.dma_start(bg2r[:], segf("bg2", [128, 1]))
                bmur = pc.tile([1, 128], f32, tag="bmu")
                nc.sync.dma_start(bmur[:], segf("bmu", [1, 128]))
                blvr = pc.tile([1, 128], f32, tag="blv")
                nc.sync.dma_start(blvr[:], segf("blv", [1, 128]))
                h3Ts = pc.tile([128, 8 * NLOC], bf, tag="h3T")
                relu1 = pc.tile([128, NLOC], bf, tag="relu1")
                eg = pc.tile([128, NTILES], bf, tag="eg")
                with tc.tile_pool(name="ppa", bufs=1, space="PSUM") as pa:
                    for t in range(NTILES):
                        for kc in range(8):
                            ptr = trp.tile([128, 128], bf, tag="tr")
                            nc.tensor.transpose(
                                out=ptr[:],
                                in_=h3[t][:, kc * 128:(kc + 1) * 128],
                                identity=ident[:])
                            nc.scalar.activation(
                                h3Ts[:, kc * NLOC + t * 128:
                                     kc * NLOC + (t + 1) * 128],
                                ptr[:], AF.Copy)
                    nwin = (NLOC + 511) // 512
                    for w in range(nwin):
                        s = w * 512
                        ww = min(512, NLOC - s)
                        g1p = pa.tile([128, 512], f32, tag="g1", bufs=2)
                        for kc in range(8):
                            nc.tensor.matmul(
                                out=g1p[:, :ww], lhsT=Wg1s[:, kc, :],
                                rhs=h3Ts[:, kc * NLOC + s:kc * NLOC + s + ww],
                                start=(kc == 0), stop=(kc == 7))
                        nc.scalar.activation(relu1[:, s:s + ww], g1p[:, :ww],
                                             AF.Relu, bias=bg1c[:])
                    gps = pa.tile([128, 32], f32, tag="g2")
                    for t in range(NTILES):
                        nc.tensor.matmul(out=gps[:, t:t + 1],
                                         lhsT=relu1[:, t * 128:(t + 1) * 128],
                                         rhs=Wg2c[:], start=True, stop=True)
                    nc.scalar.activation(eg[:], gps[:, :NTILES], AF.Exp,
                                         bias=bg2r[:])
                with tc.tile_pool(name="ppb", bufs=1, space="PSUM") as pb:
                    gd = pb.tile([GPAD, 1], f32, tag="gd")
                    geps = [pb.tile([GPAD, 512], f32, tag=f"ge{j}",
                                    name=f"geps{j}") for j in range(2)]
                    for t in range(NTILES):
                        goh = sp.tile([128, GPAD], bf, tag="goh")
                        nc.vector.tensor_tensor(
                            out=goh[:],
                            in0=gidsb[:, t:t + 1].to_broadcast([128, GPAD]),
                            in1=iota_row[:, 0:GPAD], op=ALU.is_equal)
                        nc.tensor.matmul(out=gd[:], lhsT=goh[:],
                                         rhs=eg[:, t:t + 1],
                                         start=(t == 0),
                                         stop=(t == NTILES - 1))
                        goha = sp.tile([128, GPAD], bf, tag="goha")
                        nc.vector.tensor_mul(
                            goha[:], goh[:],
                            eg[:, t:t + 1].to_broadcast([128, GPAD]))
                        for j in range(2):
                            nc.tensor.matmul(
                                out=geps[j][:], lhsT=goha[:],
                                rhs=h3[t][:, j * 512:(j + 1) * 512],
                                start=(t == 0), stop=(t == NTILES - 1))
                    gdc = sp.tile([GPAD, 1], f32, tag="gdc")
                    nc.vector.tensor_scalar_max(gdc[:], gd[:], 1e-9)
                    grc = sp.tile([GPAD, 1], f32, tag="grc")
                    nc.vector.reciprocal(grc[:], gdc[:])
                    zge = sp.tile([GPAD, D3], f32, tag="zge", bufs=1)
                    nc.vector.memset(zge[:], 0.0)
                    ge = sp.tile([GPAD, D3], f32, tag="ge", bufs=1)
                    for j in range(2):
                        nc.vector.scalar_tensor_tensor(
                            out=ge[:, j * 512:(j + 1) * 512], in0=geps[j][:],
                            scalar=grc[:, 0:1],
                            in1=zge[:, j * 512:(j + 1) * 512],
                            op0=ALU.mult, op1=ALU.add)
                    geT = pc.tile([128, 8 * GPAD], f32, tag="geT")
                    for kc in range(8):
                        pst = pb.tile([128, GPAD], f32, tag="pst", bufs=1)
                        nc.tensor.transpose(out=pst[:],
                                            in_=ge[:, kc * 128:(kc + 1) * 128],
                                            identity=ident32[:])
                        nc.vector.tensor_copy(
                            geT[:, kc * GPAD:(kc + 1) * GPAD], pst[:])
                    for Wt, bt, outp in [(Wmus, bmur, mu), (Wlvs, blvr, lv)]:
                        mps = pb.tile([GPAD, 128], f32, tag="mps", bufs=1)
                        for kc in range(8):
                            nc.tensor.matmul(
                                out=mps[:],
                                lhsT=geT[:, kc * GPAD:(kc + 1) * GPAD],
                                rhs=Wt[:, kc, :],
                                start=(kc == 0), stop=False)
                        nc.tensor.matmul(out=mps[:], lhsT=on32[:], rhs=bt[:],
                                         start=False, stop=True)
                        mo = sp.tile([GPAD, 128], f32, tag="mo")
                        nc.vector.tensor_copy(mo[:], mps[:])
                        nc.sync.dma_start(outp[:], mo[:])
    nc.compile()
    return nc


_BUILD_CACHE = {}


def _get(key, fn):
    if key not in _BUILD_CACHE:
        _BUILD_CACHE[key] = fn()
    return _BUILD_CACHE[key]


def _run(nc, in_maps):
    from concourse.bass_utils import run_bass_kernel_spmd
    return run_bass_kernel_spmd(nc, in_maps, core_ids=list(range(NCORES))).results


# ------------------------------------------------------------------ main entry
def kernel(node_feat, src, dst, graph_ids,
           W1, al1, ar1, b1, W2, al2, ar2, b2, W3, al3, ar3, b3,
           Wg1, bg1, Wg2, bg2, Wmu, bmu, Wlv, blv):
    prep = host_prep(node_feat, src, dst, graph_ids)
    wb, wf = pack_weights(
        [np.asarray(W1, np.float32), np.asarray(W2, np.float32),
         np.asarray(W3, np.float32)],
        [np.asarray(al1, np.float32), np.asarray(al2, np.float32),
         np.asarray(al3, np.float32)],
        [np.asarray(ar1, np.float32), np.asarray(ar2, np.float32),
         np.asarray(ar3, np.float32)],
        [np.asarray(b1, np.float32), np.asarray(b2, np.float32),
         np.asarray(b3, np.float32)],
        Wg1, bg1, Wg2, bg2, Wmu, bmu, Wlv, blv)
    ncF = _get(("FUSED",), build_fused)
    in_maps = [dict(x1T=prep["x1T"][c], idx=prep["idx"][c], dp=prep["dp"][c],
                    gid=prep["gid"][c], wbf=wb[c], wf32=wf[c])
               for c in range(NCORES)]
    res = _run(ncF, in_maps)
    mu = np.concatenate([res[c]["mu"][:GPC] for c in range(NCORES)], 0)
    lv = np.concatenate([res[c]["lv"][:GPC] for c in range(NCORES)], 0)
    return np.asarray(mu, np.float32), np.asarray(lv, np.float32)


# revision 21
# speedup vs baseline: 242.3011x; 6.6663x over previous
# kernel.py — CrystalGCNEncoder (3-layer GAT + global attention pooling) on 8 trn2
# NeuronCores, fully fused into ONE SPMD launch.
#
# The previous version ran 7 separate launches and shipped the replicated
# per-layer feature table (plus dense one-hot scatter matrices) from the host
# every layer — hundreds of MB over the axon tunnel per call.  This version
# keeps everything on device:
#   - each core owns 25 graphs' nodes (slots balanced by in-degree over 21
#     tiles of 128) and all edges whose dst lives there
#   - per layer: local fc matmuls (P), AllGather of the [nstar, ROW] feature
#     table into Shared DRAM, then edge gather + softmax + one-hot-matmul
#     aggregation (L) — one-hot matrices are built on device from int32
#     dst-position indices via iota + is_equal
#   - weights ship sharded (1/8th per core) and are AllGathered on device
#   - pooling/readout fully local; host only concatenates the [32,128] outputs
# H2D is ~11 MB total instead of ~1.5 GB.
import numpy as np
import ml_dtypes

N, E, G = 20000, 320000, 200
F_IN, HID, H, LAT = 128, 128, 4, 128
O1, O2, O3 = HID // 2, HID, 2 * HID
D1, D2, D3 = H * O1, H * O2, H * O3          # 256, 512, 1024
NEG_SLOPE = 0.2
NCORES = 8
BF16 = ml_dtypes.bfloat16

NTILES = 21
NLOC = NTILES * 128          # 2688 slots per core
NSTAR = NLOC * NCORES        # 21504
CPT = 16                     # edge chunks (of 128) per tile
TPE = CPT * 128              # 2048 edges per tile
ELOC = NTILES * TPE          # 43008 edge slots per core
NCH = ELOC // 128            # 336 chunks per core
GPC = G // NCORES            # 25 graphs per core
GPAD = 32

LAYERS = [(F_IN, D1), (D1, D2), (D2, D3)]    # (Din, Dout)


def _row_elems(d):          # feat row: [d feats | 4 el | 4 ee | pad] bf16
    b = (d + 8) * 2
    return ((b + 255) // 256 * 256) // 2


ROWS = [_row_elems(d) for _, d in LAYERS]    # 384, 640, 1152


def _colchunks(c):
    out, s = [], 0
    while s < c:
        w = min(512, c - s)
        out.append((s, w))
        s += w
    return out


# ------------------------------------------------------------- weight layout
def _wlayout():
    """(bf16 offsets, bf16 total, f32 offsets, f32 total) of the packed,
    core-sharded weight buffers."""
    offb, ob = {}, 0
    for nm, n in [("Wa1", 128 * 1 * (D1 + 8)), ("Wa2", 128 * 2 * (D2 + 8)),
                  ("Wa3", 128 * 4 * (D3 + 8)), ("b1", D1), ("b2", D2),
                  ("b3", D3), ("Wg1", 128 * 8 * 128), ("Wg2", 128)]:
        offb[nm] = ob
        ob += n
    offf, of = {}, 0
    for nm, n in [("Wmu", 128 * 8 * 128), ("Wlv", 128 * 8 * 128),
                  ("bg1", 128), ("bg2", 128), ("bmu", 128), ("blv", 128)]:
        offf[nm] = of
        of += n
    pb = -(-ob // NCORES) * NCORES
    pf = -(-of // NCORES) * NCORES
    return offb, pb, offf, pf


OFFB, TOTB, OFFF, TOTF = _wlayout()
PB, PF = TOTB // NCORES, TOTF // NCORES


# ------------------------------------------------------------------ host prep
def host_prep(node_feat, src, dst, graph_ids):
    node_feat = np.asarray(node_feat, np.float32)
    src = np.asarray(src).astype(np.int64)
    dst = np.asarray(dst).astype(np.int64)
    graph_ids = np.asarray(graph_ids).astype(np.int64)

    gbounds = np.arange(NCORES + 1) * GPC
    nbounds = np.searchsorted(graph_ids, gbounds)
    core_of_node = np.searchsorted(nbounds, np.arange(N), side="right") - 1
    indeg = np.bincount(dst, minlength=N)

    glob2slot = np.zeros(N, np.int64)
    tile_of_node = np.zeros(N, np.int64)
    slotpos_of_node = np.zeros(N, np.int64)
    for c in range(NCORES):
        nodes = np.arange(nbounds[c], nbounds[c + 1])
        assert len(nodes) <= NLOC
        order = nodes[np.argsort(-indeg[nodes], kind="stable")]
        loads = np.zeros(NTILES, np.int64)
        counts = np.zeros(NTILES, np.int64)
        for nd in order:
            free = np.nonzero(counts < 128)[0]
            tgt = free[np.argmin(loads[free])]
            tile_of_node[nd] = tgt
            slotpos_of_node[nd] = counts[tgt]
            glob2slot[nd] = c * NLOC + tgt * 128 + counts[tgt]
            counts[tgt] += 1
            loads[tgt] += indeg[nd]
        assert loads.max() <= TPE

    edge_core = core_of_node[dst]
    idx_l, dp_l, gid_l = [], [], []
    for c in range(NCORES):
        eids = np.nonzero(edge_core == c)[0]
        assert len(eids) <= ELOC
        src_slot = np.zeros(ELOC, np.int64)
        dst_pos = np.full(ELOC, -1, np.int64)
        et = tile_of_node[dst[eids]]
        for t in range(NTILES):
            sel = eids[et == t]
            assert len(sel) <= TPE
            b = t * TPE
            src_slot[b : b + len(sel)] = glob2slot[src[sel]]
            dst_pos[b : b + len(sel)] = slotpos_of_node[dst[sel]]
        idx_l.append(np.ascontiguousarray(
            src_slot.reshape(NCH, 128).T).astype(np.int32))
        dp_l.append(np.ascontiguousarray(
            dst_pos.reshape(NCH, 128).T).astype(np.int32))
        gid = np.full((128, NTILES), -1, np.int64)
        nodes = np.arange(nbounds[c], nbounds[c + 1])
        gid[slotpos_of_node[nodes], tile_of_node[nodes]] = \
            graph_ids[nodes] - c * GPC
        gid_l.append(gid.astype(np.int32))

    x1 = np.zeros((NSTAR, F_IN), np.float32)
    x1[glob2slot] = node_feat
    x1T_l = [np.ascontiguousarray(x1[c * NLOC:(c + 1) * NLOC].T).astype(BF16)
             for c in range(NCORES)]
    return dict(idx=idx_l, dp=dp_l, gid=gid_l, x1T=x1T_l)


def fold_weights(W, al, ar):
    """[Din, Dout] (+attn vecs) -> [Din, Dout+8] = [W | W@al_h | W@ar_h]."""
    Din, D = W.shape
    Hh, O = al.shape
    Wl = np.einsum("iho,ho->ih", W.reshape(Din, Hh, O), al)
    Wr = np.einsum("iho,ho->ih", W.reshape(Din, Hh, O), ar)
    return np.concatenate([W, Wl, Wr], 1)


def wstack_flat(Waug):
    """[Din, C] -> flat (p, k, c)-major [128 * Din/128 * C]."""
    Din, C = Waug.shape
    return np.ascontiguousarray(
        Waug.reshape(Din // 128, 128, C).transpose(1, 0, 2)).reshape(-1)


def pack_weights(Ws, als, ars, bs, Wg1, bg1, Wg2, bg2, Wmu, bmu, Wlv, blv):
    bfb = np.zeros(TOTB, BF16)
    def putb(nm, a):
        a = np.asarray(a, np.float32).reshape(-1)
        bfb[OFFB[nm]:OFFB[nm] + len(a)] = a.astype(BF16)
    for i in range(3):
        putb(f"Wa{i+1}", wstack_flat(fold_weights(Ws[i], als[i], ars[i])))
        putb(f"b{i+1}", bs[i])
    putb("Wg1", wstack_flat(np.asarray(Wg1, np.float32)))
    putb("Wg2", Wg2)
    f32b = np.zeros(TOTF, np.float32)
    def putf(nm, a):
        a = np.asarray(a, np.float32).reshape(-1)
        f32b[OFFF[nm]:OFFF[nm] + len(a)] = a
    putf("Wmu", wstack_flat(np.asarray(Wmu, np.float32)))
    putf("Wlv", wstack_flat(np.asarray(Wlv, np.float32)))
    putf("bg1", bg1)
    putf("bg2", np.full(128, np.asarray(bg2, np.float32).reshape(-1)[0]))
    putf("bmu", bmu)
    putf("blv", blv)
    return bfb.reshape(NCORES, PB), f32b.reshape(NCORES, PF)


# ------------------------------------------------------------------ builder
def build_fused():
    import concourse.bass as bass
    import concourse.tile as tile
    from concourse import bacc, mybir

    bf = mybir.dt.bfloat16
    f32 = mybir.dt.float32
    i32 = mybir.dt.int32
    AF = mybir.ActivationFunctionType
    ALU = mybir.AluOpType
    RG = [list(range(NCORES))]

    nc = bacc.Bacc("TRN2", target_bir_lowering=False, debug=False,
                   num_devices=NCORES)
    x1T = nc.dram_tensor("x1T", [128, NLOC], bf, kind="ExternalInput").ap()
    idx = nc.dram_tensor("idx", [128, NCH], i32, kind="ExternalInput").ap()
    dp = nc.dram_tensor("dp", [128, NCH], i32, kind="ExternalInput").ap()
    gid = nc.dram_tensor("gid", [128, NTILES], i32, kind="ExternalInput").ap()
    wbf = nc.dram_tensor("wbf", [PB], bf, kind="ExternalInput").ap()
    wf32 = nc.dram_tensor("wf32", [PF], f32, kind="ExternalInput").ap()
    mu = nc.dram_tensor("mu", [GPAD, 128], f32, kind="ExternalOutput").ap()
    lv = nc.dram_tensor("lv", [GPAD, 128], f32, kind="ExternalOutput").ap()

    wbl = nc.dram_tensor("wbl", [PB], bf, kind="Internal").ap()
    wbs = nc.dram_tensor("wbs", [TOTB], bf, kind="Internal",
                         addr_space="Shared").ap()
    wfl = nc.dram_tensor("wfl", [PF], f32, kind="Internal").ap()
    wfs = nc.dram_tensor("wfs", [TOTF], f32, kind="Internal",
                         addr_space="Shared").ap()
    flocs, ftabs = [], []
    for l in range(3):
        flocs.append(nc.dram_tensor(f"floc{l}", [NLOC, ROWS[l]], bf,
                                    kind="Internal").ap())
        ftabs.append(nc.dram_tensor(f"ftab{l}", [NSTAR, ROWS[l]], bf,
                                    kind="Internal", addr_space="Shared").ap())

    with tile.TileContext(nc) as tc:
        with tc.tile_pool(name="cst", bufs=1) as cp, \
             tc.tile_pool(name="xper", bufs=1) as xp, \
             tc.tile_pool(name="trp", bufs=2, space="PSUM") as trp:
            # ---- stage + allgather weights
            nc.sync.dma_start(wbl[:], wbf[:])
            nc.sync.dma_start(wfl[:], wf32[:])
            nc.gpsimd.collective_compute(
                "AllGather", ALU.bypass, replica_groups=RG,
                ins=[wbl[:]], outs=[wbs[:]])
            nc.gpsimd.collective_compute(
                "AllGather", ALU.bypass, replica_groups=RG,
                ins=[wfl[:]], outs=[wfs[:]])

            def segb(nm, shape):
                n = int(np.prod(shape))
                a = wbs[OFFB[nm]:OFFB[nm] + n]
                if len(shape) == 3:
                    return a.rearrange("(p k c) -> p k c", p=shape[0],
                                       k=shape[1], c=shape[2])
                return a.rearrange("(a c) -> a c", a=shape[0], c=shape[1])

            def segf(nm, shape):
                n = int(np.prod(shape))
                a = wfs[OFFF[nm]:OFFF[nm] + n]
                if len(shape) == 3:
                    return a.rearrange("(p k c) -> p k c", p=shape[0],
                                       k=shape[1], c=shape[2])
                return a.rearrange("(a c) -> a c", a=shape[0], c=shape[1])

            # ---- constants
            iota_row = cp.tile([128, 128], i32, tag="io_r")
            nc.gpsimd.iota(iota_row[:], pattern=[[1, 128]], base=0,
                           channel_multiplier=0)
            iota_col = cp.tile([128, 128], i32, tag="io_c")
            nc.gpsimd.iota(iota_col[:], pattern=[[0, 128]], base=0,
                           channel_multiplier=1)
            iota16 = iota_row[:].unsqueeze(1).broadcast_to([128, CPT, 128])
            ident = cp.tile([128, 128], bf, tag="ident")
            nc.vector.tensor_tensor(out=ident[:], in0=iota_row[:],
                                    in1=iota_col[:], op=ALU.is_equal)
            ident32 = cp.tile([32, 32], f32, tag="id32")
            nc.vector.tensor_tensor(out=ident32[:], in0=iota_row[0:32, 0:32],
                                    in1=iota_col[0:32, 0:32], op=ALU.is_equal)
            on1 = cp.tile([1, 128], bf, tag="on1")
            nc.vector.memset(on1[:], 1.0)
            on32 = cp.tile([1, 32], f32, tag="on32")
            nc.vector.memset(on32[:], 1.0)

            idxsb = cp.tile([128, NCH], i32, tag="idx")
            nc.sync.dma_start(idxsb[:], idx[:])
            dpsb = cp.tile([128, NCH], i32, tag="dp")
            nc.sync.dma_start(dpsb[:], dp[:])
            gidsb = cp.tile([128, NTILES], i32, tag="gid")
            nc.sync.dma_start(gidsb[:], gid[:])

            xtiles = {1: [], 2: [], 3: []}   # layer outputs [128, Dout]/tile

            # ================================================== layers
            for l, (Din, Dout) in enumerate(LAYERS):
                K, O, ROW = Din // 128, Dout // H, ROWS[l]
                fcks = _colchunks(Dout + 8)
                with tc.tile_pool(name=f"l{l}c", bufs=1) as lc:
                    ers = lc.tile([128, NTILES * 4], bf, tag="ers")
                    bsb = lc.tile([128, Dout], bf, tag="bsb")
                    Wa = lc.tile([128, K, Dout + 8], bf, tag="Wa")
                    nc.sync.dma_start(Wa[:], segb(f"Wa{l+1}",
                                                  [128, K, Dout + 8]))
                    brow = lc.tile([1, Dout], bf, tag="br")
                    nc.sync.dma_start(brow[:], segb(f"b{l+1}", [1, Dout]))
                    if l == 0:
                        x1Ts = lc.tile([128, NLOC], bf, tag="x1T")
                        nc.sync.dma_start(x1Ts[:], x1T[:])
                    # ---------- P phase: feat/el/er matmuls on own nodes
                    with tc.tile_pool(name=f"P{l}s", bufs=3) as sp, \
                         tc.tile_pool(name=f"P{l}p", bufs=1,
                                      space="PSUM") as pp:
                        bps = pp.tile([128, Dout], f32, tag="bps")
                        for (s, w) in _colchunks(Dout):
                            nc.tensor.matmul(out=bps[:, s:s + w],
                                             lhsT=on1[:],
                                             rhs=brow[:, s:s + w],
                                             start=True, stop=True)
                        nc.vector.tensor_copy(bsb[:], bps[:])
                        for t in range(NTILES):
                            pas = [pp.tile([128, w], f32, tag=f"pa{j}",
                                           name=f"pa{j}")
                                   for j, (s, w) in enumerate(fcks)]
                            for kc in range(K):
                                if l == 0:
                                    xt = x1Ts[:, t * 128:(t + 1) * 128]
                                else:
                                    ptr = trp.tile([128, 128], bf, tag="tr")
                                    nc.tensor.transpose(
                                        out=ptr[:],
                                        in_=xtiles[l][t][:, kc * 128:
                                                         (kc + 1) * 128],
                                        identity=ident[:])
                                    xts = sp.tile([128, 128], bf, tag="xt")
                                    nc.scalar.activation(xts[:], ptr[:],
                                                         AF.Copy)
                                    xt = xts[:]
                                for j, (s, w) in enumerate(fcks):
                                    nc.tensor.matmul(
                                        out=pas[j][:], lhsT=xt,
                                        rhs=Wa[:, kc, s:s + w],
                                        start=(kc == 0), stop=(kc == K - 1))
                            ft = sp.tile([128, ROW], bf, tag="ft")
                            for j, (s, w) in enumerate(fcks):
                                fw = min(s + w, Dout + 4) - s
                                if fw > 0:
                                    nc.vector.tensor_copy(ft[:, s:s + fw],
                                                          pas[j][:, 0:fw])
                            nc.vector.tensor_copy(
                                ers[:, t * 4:(t + 1) * 4],
                                pas[-1][:, fcks[-1][1] - 4:fcks[-1][1]])
                            nc.sync.dma_start(
                                flocs[l][t * 128:(t + 1) * 128, :], ft[:])
                    # ---------- AllGather feature table
                    nc.gpsimd.collective_compute(
                        "AllGather", ALU.bypass, replica_groups=RG,
                        ins=[flocs[l][:]], outs=[ftabs[l][:]])
                    # ---------- L phase: gather, edge softmax, aggregate
                    with tc.tile_pool(name=f"L{l}g", bufs=2) as gp, \
                         tc.tile_pool(name=f"L{l}o", bufs=2) as ohp, \
                         tc.tile_pool(name=f"L{l}t", bufs=3) as otp, \
                         tc.tile_pool(name=f"L{l}s", bufs=2) as sp, \
                         tc.tile_pool(name=f"L{l}p", bufs=1,
                                      space="PSUM") as pp:
                        for t in range(NTILES):
                            gtt = gp.tile([128, CPT * ROW], bf, tag="g")
                            oh16 = ohp.tile([128, CPT * 128], bf, tag="oh")
                            nc.vector.tensor_tensor(
                                out=oh16[:].rearrange("p (c e) -> p c e",
                                                      c=CPT, e=128),
                                in0=dpsb[:, t * CPT:(t + 1) * CPT]
                                    .to_broadcast([128, CPT, 128]),
                                in1=iota16, op=ALU.is_equal)
                            erps = pp.tile([128, CPT * 4], f32, tag="erps")
                            for c in range(CPT):
                                ch = t * CPT + c
                                nc.gpsimd.indirect_dma_start(
                                    out=gtt[:, c * ROW:(c + 1) * ROW],
                                    out_offset=None, in_=ftabs[l][:],
                                    in_offset=bass.IndirectOffsetOnAxis(
                                        ap=idxsb[:, ch:ch + 1], axis=0))
                                ptr = trp.tile([128, 128], bf, tag="tr")
                                nc.tensor.transpose(
                                    out=ptr[:],
                                    in_=oh16[:, c * 128:(c + 1) * 128],
                                    identity=ident[:])
                                oht = otp.tile([128, 128], bf, tag="oht")
                                nc.scalar.activation(oht[:], ptr[:], AF.Copy)
                                nc.tensor.matmul(
                                    out=erps[:, c * 4:(c + 1) * 4],
                                    lhsT=oht[:],
                                    rhs=ers[:, t * 4:(t + 1) * 4],
                                    start=True, stop=True)
                            g3 = gtt[:].rearrange("p (c r) -> p c r",
                                                  c=CPT, r=ROW)
                            zz = sp.tile([128, CPT * 4], f32, tag="zz")
                            nc.vector.tensor_add(
                                zz[:].rearrange("p (c h) -> p c h",
                                                c=CPT, h=4),
                                g3[:, :, Dout:Dout + 4],
                                erps[:].rearrange("p (c h) -> p c h",
                                                  c=CPT, h=4))
                            za = sp.tile([128, CPT * 4], f32, tag="za")
                            nc.vector.scalar_tensor_tensor(
                                out=za[:], in0=zz[:], scalar=NEG_SLOPE,
                                in1=zz[:], op0=ALU.mult, op1=ALU.max)
                            ee = sp.tile([128, CPT * 4], bf, tag="ee")
                            nc.scalar.activation(ee[:], za[:], AF.Exp)
                            # ee into the row tail (feeds the den columns)
                            nc.vector.tensor_copy(
                                g3[:, :, Dout + 4:Dout + 8],
                                ee[:].rearrange("p (c h) -> p c h",
                                                c=CPT, h=4))
                            rstps = [pp.tile([128, w], f32, tag=f"rst{j}",
                                             name=f"rst{j}")
                                     for j, (s, w) in enumerate(fcks)]
                            for c in range(CPT):
                                e4 = ee[:, c * 4:(c + 1) * 4]
                                gslc = gtt[:, c * ROW:c * ROW + Dout] \
                                    .rearrange("p (h o) -> p h o", h=H, o=O)
                                nc.vector.scalar_tensor_tensor(
                                    out=gslc, in0=gslc, scalar=1.0,
                                    in1=e4.to_broadcast([128, H, O]),
                                    op0=ALU.mult, op1=ALU.mult)
                                for j, (s, w) in enumerate(fcks):
                                    nc.tensor.matmul(
                                        out=rstps[j][:],
                                        lhsT=oh16[:, c * 128:(c + 1) * 128],
                                        rhs=gtt[:, c * ROW + s:
                                                c * ROW + s + w],
                                        start=(c == 0), stop=(c == CPT - 1))
                            wl = fcks[-1][1]
                            dcl = sp.tile([128, 4], f32, tag="dcl")
                            nc.vector.tensor_scalar_max(
                                dcl[:], rstps[-1][:, wl - 4:wl], 1e-9)
                            rec = sp.tile([128, 4], f32, tag="rec")
                            nc.vector.reciprocal(rec[:], dcl[:])
                            y = sp.tile([128, Dout], f32, tag="y", bufs=1)
                            for h in range(H):
                                j = (h * O) // 512
                                s0 = (h * O) % 512
                                nc.vector.scalar_tensor_tensor(
                                    out=y[:, h * O:(h + 1) * O],
                                    in0=rstps[j][:, s0:s0 + O],
                                    scalar=rec[:, h:h + 1],
                                    in1=bsb[:, h * O:(h + 1) * O],
                                    op0=ALU.mult, op1=ALU.add)
                            mn = sp.tile([128, Dout], f32, tag="mn", bufs=1)
                            nc.vector.tensor_scalar_min(mn[:], y[:], 0.0)
                            nc.scalar.activation(mn[:], mn[:], AF.Exp)
                            nc.vector.scalar_tensor_tensor(
                                out=y[:], in0=y[:], scalar=0.0, in1=mn[:],
                                op0=ALU.max, op1=ALU.add)
                            xo = xp.tile([128, Dout], bf, tag=f"xo{l}_{t}",
                                         name=f"xo{l}_{t}")
                            nc.vector.tensor_scalar_add(xo[:], y[:], -1.0)
                            xtiles[l + 1].append(xo)

            # ================================================== pooling
            h3 = xtiles[3]
            with tc.tile_pool(name="pc", bufs=1) as pc, \
                 tc.tile_pool(name="ps", bufs=3) as sp:
                Wg1s = pc.tile([128, 8, 128], bf, tag="Wg1")
                nc.sync.dma_start(Wg1s[:], segb("Wg1", [128, 8, 128]))
                Wg2c = pc.tile([128, 1], bf, tag="Wg2")
                nc.sync.dma_start(Wg2c[:], segb("Wg2", [128, 1]))
                Wmus = pc.tile([128, 8, 128], f32, tag="Wmu")
                nc.sync.dma_start(Wmus[:], segf("Wmu", [128, 8, 128]))
                Wlvs = pc.tile([128, 8, 128], f32, tag="Wlv")
                nc.sync.dma_start(Wlvs[:], segf("Wlv", [128, 8, 128]))
                bg1c = pc.tile([128, 1], f32, tag="bg1")
                nc.sync.dma_start(bg1c[:], segf("bg1", [128, 1]))
                bg2r = pc.tile([128, 1], f32, tag="bg2")
                nc.sync.dma_start(bg2r[:], segf("bg2", [128, 1]))
                bmur = pc.tile([1, 128], f32, tag="bmu")
                nc.sync.dma_start(bmur[:], segf("bmu", [1, 128]))
                blvr = pc.tile([1, 128], f32, tag="blv")
                nc.sync.dma_start(blvr[:], segf("blv", [1, 128]))
                h3Ts = pc.tile([128, 8 * NLOC], bf, tag="h3T")
                relu1 = pc.tile([128, NLOC], bf, tag="relu1")
                eg = pc.tile([128, NTILES], bf, tag="eg")
                with tc.tile_pool(name="ppa", bufs=1, space="PSUM") as pa:
                    for t in range(NTILES):
                        for kc in range(8):
                            ptr = trp.tile([128, 128], bf, tag="tr")
                            nc.tensor.transpose(
                                out=ptr[:],
                                in_=h3[t][:, kc * 128:(kc + 1) * 128],
                                identity=ident[:])
                            nc.scalar.activation(
                                h3Ts[:, kc * NLOC + t * 128:
                                     kc * NLOC + (t + 1) * 128],
                                ptr[:], AF.Copy)
                    nwin = (NLOC + 511) // 512
                    for w in range(nwin):
                        s = w * 512
                        ww = min(512, NLOC - s)
                        g1p = pa.tile([128, 512], f32, tag="g1", bufs=2)
                        for kc in range(8):
                            nc.tensor.matmul(
                                out=g1p[:, :ww], lhsT=Wg1s[:, kc, :],
                                rhs=h3Ts[:, kc * NLOC + s:kc * NLOC + s + ww],
                                start=(kc == 0), stop=(kc == 7))
                        nc.scalar.activation(relu1[:, s:s + ww], g1p[:, :ww],
                                             AF.Relu, bias=bg1c[:])
                    gps = pa.tile([128, 32], f32, tag="g2")
                    for t in range(NTILES):
                        nc.tensor.matmul(out=gps[:, t:t + 1],
                                         lhsT=relu1[:, t * 128:(t + 1) * 128],
                                         rhs=Wg2c[:], start=True, stop=True)
                    nc.scalar.activation(eg[:], gps[:, :NTILES], AF.Exp,
                                         bias=bg2r[:])
                with tc.tile_pool(name="ppb", bufs=1, space="PSUM") as pb:
                    gd = pb.tile([GPAD, 1], f32, tag="gd")
                    geps = [pb.tile([GPAD, 512], f32, tag=f"ge{j}",
                                    name=f"geps{j}") for j in range(2)]
                    for t in range(NTILES):
                        goh = sp.tile([128, GPAD], bf, tag="goh")
                        nc.vector.tensor_tensor(
                            out=goh[:],
                            in0=gidsb[:, t:t + 1].to_broadcast([128, GPAD]),
                            in1=iota_row[:, 0:GPAD], op=ALU.is_equal)
                        nc.tensor.matmul(out=gd[:], lhsT=goh[:],
                                         rhs=eg[:, t:t + 1],
                                         start=(t == 0),
                                         stop=(t == NTILES - 1))
                        goha = sp.tile([128, GPAD], bf, tag="goha")
                        nc.vector.tensor_mul(
                            goha[:], goh[:],
                            eg[:, t:t + 1].to_broadcast([128, GPAD]))
                        for j in range(2):
                            nc.tensor.matmul(
                                out=geps[j][:], lhsT=goha[:],
                                rhs=h3[t][:, j * 512:(j + 1) * 512],
                                start=(t == 0), stop=(t == NTILES - 1))
                    gdc = sp.tile([GPAD, 1], f32, tag="gdc")
                    nc.vector.tensor_scalar_max(gdc[:], gd[:], 1e-9)
                    grc = sp.tile([GPAD, 1], f32, tag="grc")
                    nc.vector.reciprocal(grc[:], gdc[:])
                    zge = sp.tile([GPAD, D3], f32, tag="zge", bufs=1)
                    nc.vector.memset(zge[:], 0.0)
                    ge = sp.tile([GPAD, D3], f32, tag="ge", bufs=1)
                    for j in range(2):
                        nc.vector.scalar_tensor_tensor(
                            out=ge[:, j * 512:(j + 1) * 512], in0=geps[j][:],
                            scalar=grc[:, 0:1],
                            in1=zge[:, j * 512:(j + 1) * 512],
                            op0=ALU.mult, op1=ALU.add)
                    geT = pc.tile([128, 8 * GPAD], f32, tag="geT")
                    for kc in range(8):
                        pst = pb.tile([128, GPAD], f32, tag="pst", bufs=1)
                        nc.tensor.transpose(out=pst[:],
                                            in_=ge[:, kc * 128:(kc + 1) * 128],
                                            identity=ident32[:])
                        nc.vector.tensor_copy(
                            geT[:, kc * GPAD:(kc + 1) * GPAD], pst[:])
                    for Wt, bt, outp in [(Wmus, bmur, mu), (Wlvs, blvr, lv)]:
                        mps = pb.tile([GPAD, 128], f32, tag="mps", bufs=1)
                        for kc in range(8):
                            nc.tensor.matmul(
                                out=mps[:],
                                lhsT=geT[:, kc * GPAD:(kc + 1) * GPAD],
                                rhs=Wt[:, kc, :],
                                start=(kc == 0), stop=False)
                        nc.tensor.matmul(out=mps[:], lhsT=on32[:], rhs=bt[:],
                                         start=False, stop=True)
                        mo = sp.tile([GPAD, 128], f32, tag="mo")
                        nc.vector.tensor_copy(mo[:], mps[:])
                        nc.sync.dma_start(outp[:], mo[:])
    nc.compile()
    return nc


_BUILD_CACHE = {}


def _get(key, fn):
    if key not in _BUILD_CACHE:
        _BUILD_CACHE[key] = fn()
    return _BUILD_CACHE[key]


def _make_runner(nc):
    """Build the PJRT executable for ``nc`` once and reuse it across calls —
    the stock run_bass_kernel_spmd re-jits (and re-lowers through XLA) on
    every invocation."""
    import jax
    import numpy as _np
    from jax.sharding import Mesh, PartitionSpec
    from jax.experimental.shard_map import shard_map
    from concourse import bass2jax, mybir

    bass2jax.install_neuronx_cc_hook()
    partition_name = (nc.partition_id_tensor.name
                      if nc.partition_id_tensor else None)
    in_names, out_names, out_avals, zero_outs = [], [], [], []
    for alloc in nc.m.functions[0].allocations:
        if not isinstance(alloc, mybir.MemoryLocationSet):
            continue
        name = alloc.memorylocations[0].name
        if alloc.kind == "ExternalInput":
            if name != partition_name:
                in_names.append(name)
        elif alloc.kind == "ExternalOutput":
            shape = tuple(alloc.tensor_shape)
            dtype = mybir.dt.np(alloc.dtype)
            out_names.append(name)
            out_avals.append(jax.core.ShapedArray(shape, dtype))
            zero_outs.append(_np.zeros(shape, dtype))
    n_params, n_outs = len(in_names), len(out_avals)
    in_names = in_names + out_names
    if partition_name is not None:
        in_names.append(partition_name)
    donate = tuple(range(n_params, n_params + n_outs))

    def _body(*args):
        operands = list(args)
        if partition_name is not None:
            operands.append(bass2jax.partition_id_tensor())
        outs = bass2jax._bass_exec_p.bind(
            *operands, out_avals=tuple(out_avals), in_names=tuple(in_names),
            out_names=tuple(out_names), lowering_input_output_aliases=(),
            sim_require_finite=True, sim_require_nnan=True, nc=nc)
        return tuple(outs)

    devices = jax.devices()[:NCORES]
    mesh = Mesh(np.asarray(devices), ("core",))
    sharded = jax.jit(
        shard_map(_body, mesh=mesh,
                  in_specs=(PartitionSpec("core"),) * (n_params + n_outs),
                  out_specs=(PartitionSpec("core"),) * n_outs,
                  check_rep=False),
        donate_argnums=donate, keep_unused=True)
    return dict(fn=sharded, in_names=in_names[:n_params],
                out_names=out_names, out_avals=out_avals,
                zero_outs=zero_outs, mesh=mesh)


_DEVPUT_CACHE = {}


def _stage_input(nm, concat, mesh):
    """Upload ``concat`` sharded by core, reusing the device copy when the
    content is unchanged since the previous call (weights / graph structure
    normally are).  Content is verified host-side with memcmp."""
    import jax
    from jax.sharding import NamedSharding, PartitionSpec
    ent = _DEVPUT_CACHE.get(nm)
    if ent is not None and ent[0].shape == concat.shape \
            and ent[0].dtype == concat.dtype and np.array_equal(ent[0], concat):
        return ent[1]
    arr = jax.device_put(
        concat, NamedSharding(mesh, PartitionSpec("core")))
    _DEVPUT_CACHE[nm] = (concat, arr)
    return arr


def _run(nc, in_maps):
    import time as _time
    dbg = bool(__import__("os").environ.get("KERNEL_DEBUG_TIMING"))
    r = _get(("RUNNER", id(nc)), lambda: _make_runner(nc))
    t0 = _time.time()
    concat_in = [
        _stage_input(nm,
                     np.concatenate([np.asarray(m[nm]) for m in in_maps], 0),
                     r["mesh"])
        for nm in r["in_names"]]
    concat_zeros = [np.zeros((NCORES * z.shape[0], *z.shape[1:]), z.dtype)
                    for z in r["zero_outs"]]
    t1 = _time.time()
    out_arrs = r["fn"](*concat_in, *concat_zeros)
    out_np = [np.asarray(a) for a in out_arrs]
    t2 = _time.time()
    if dbg:
        print(f"[timing] stage-in {t1-t0:.3f}s  dispatch+exec+D2H "
              f"{t2-t1:.3f}s")
    return [
        {nm: out_np[i].reshape(NCORES, *r["out_avals"][i].shape)[c]
         for i, nm in enumerate(r["out_names"])}
        for c in range(NCORES)
    ]


# ------------------------------------------------------------------ main entry
def kernel(node_feat, src, dst, graph_ids,
           W1, al1, ar1, b1, W2, al2, ar2, b2, W3, al3, ar3, b3,
           Wg1, bg1, Wg2, bg2, Wmu, bmu, Wlv, blv):
    prep = host_prep(node_feat, src, dst, graph_ids)
    wb, wf = pack_weights(
        [np.asarray(W1, np.float32), np.asarray(W2, np.float32),
         np.asarray(W3, np.float32)],
        [np.asarray(al1, np.float32), np.asarray(al2, np.float32),
         np.asarray(al3, np.float32)],
        [np.asarray(ar1, np.float32), np.asarray(ar2, np.float32),
         np.asarray(ar3, np.float32)],
        [np.asarray(b1, np.float32), np.asarray(b2, np.float32),
         np.asarray(b3, np.float32)],
        Wg1, bg1, Wg2, bg2, Wmu, bmu, Wlv, blv)
    ncF = _get(("FUSED",), build_fused)
    in_maps = [dict(x1T=prep["x1T"][c], idx=prep["idx"][c], dp=prep["dp"][c],
                    gid=prep["gid"][c], wbf=wb[c], wf32=wf[c])
               for c in range(NCORES)]
    res = _run(ncF, in_maps)
    mu = np.concatenate([res[c]["mu"][:GPC] for c in range(NCORES)], 0)
    lv = np.concatenate([res[c]["lv"][:GPC] for c in range(NCORES)], 0)
    return np.asarray(mu, np.float32), np.asarray(lv, np.float32)


# revision 28
# speedup vs baseline: 590.5843x; 2.4374x over previous
# kernel.py — CrystalGCNEncoder (3-layer GAT + global attention pooling) on 8 trn2
# NeuronCores, fully fused into ONE SPMD launch.
#
# The previous version ran 7 separate launches and shipped the replicated
# per-layer feature table (plus dense one-hot scatter matrices) from the host
# every layer — hundreds of MB over the axon tunnel per call.  This version
# keeps everything on device:
#   - each core owns 25 graphs' nodes (slots balanced by in-degree over 21
#     tiles of 128) and all edges whose dst lives there
#   - per layer: local fc matmuls (P), AllGather of the [nstar, ROW] feature
#     table into Shared DRAM, then edge gather + softmax + one-hot-matmul
#     aggregation (L) — one-hot matrices are built on device from int32
#     dst-position indices via iota + is_equal
#   - weights ship sharded (1/8th per core) and are AllGathered on device
#   - pooling/readout fully local; host only concatenates the [32,128] outputs
# H2D is ~11 MB total instead of ~1.5 GB.
import numpy as np
import ml_dtypes

N, E, G = 20000, 320000, 200
F_IN, HID, H, LAT = 128, 128, 4, 128
O1, O2, O3 = HID // 2, HID, 2 * HID
D1, D2, D3 = H * O1, H * O2, H * O3          # 256, 512, 1024
NEG_SLOPE = 0.2
NCORES = 8
BF16 = ml_dtypes.bfloat16

NTILES = 21
NLOC = NTILES * 128          # 2688 slots per core
NSTAR = NLOC * NCORES        # 21504
CPT = 16                     # edge chunks (of 128) per tile
TPE = CPT * 128              # 2048 edges per tile
ELOC = NTILES * TPE          # 43008 edge slots per core
NCH = ELOC // 128            # 336 chunks per core
GPC = G // NCORES            # 25 graphs per core
GPAD = 32

LAYERS = [(F_IN, D1), (D1, D2), (D2, D3)]    # (Din, Dout)


def _row_elems(d):          # feat row: [d feats | 4 el | 4 ee | pad] bf16
    b = (d + 8) * 2
    return ((b + 255) // 256 * 256) // 2


ROWS = [_row_elems(d) for _, d in LAYERS]    # 384, 640, 1152


def _colchunks(c):
    out, s = [], 0
    while s < c:
        w = min(512, c - s)
        out.append((s, w))
        s += w
    return out


# ------------------------------------------------------------- weight layout
def _wlayout():
    """(bf16 offsets, bf16 total, f32 offsets, f32 total) of the packed,
    core-sharded weight buffers."""
    offb, ob = {}, 0
    for nm, n in [("Wa1", 128 * 1 * (D1 + 8)), ("Wa2", 128 * 2 * (D2 + 8)),
                  ("Wa3", 128 * 4 * (D3 + 8)), ("b1", D1), ("b2", D2),
                  ("b3", D3), ("Wg1", 128 * 8 * 128), ("Wg2", 128)]:
        offb[nm] = ob
        ob += n
    offf, of = {}, 0
    for nm, n in [("Wmu", 128 * 8 * 128), ("Wlv", 128 * 8 * 128),
                  ("bg1", 128), ("bg2", 128), ("bmu", 128), ("blv", 128)]:
        offf[nm] = of
        of += n
    pb = -(-ob // NCORES) * NCORES
    pf = -(-of // NCORES) * NCORES
    return offb, pb, offf, pf


OFFB, TOTB, OFFF, TOTF = _wlayout()
PB, PF = TOTB // NCORES, TOTF // NCORES


# ------------------------------------------------------------------ host prep
def host_prep(node_feat, src, dst, graph_ids):
    node_feat = np.asarray(node_feat, np.float32)
    src = np.asarray(src).astype(np.int64)
    dst = np.asarray(dst).astype(np.int64)
    graph_ids = np.asarray(graph_ids).astype(np.int64)

    gbounds = np.arange(NCORES + 1) * GPC
    nbounds = np.searchsorted(graph_ids, gbounds)
    core_of_node = np.searchsorted(nbounds, np.arange(N), side="right") - 1
    indeg = np.bincount(dst, minlength=N)

    glob2slot = np.zeros(N, np.int64)
    tile_of_node = np.zeros(N, np.int64)
    slotpos_of_node = np.zeros(N, np.int64)
    for c in range(NCORES):
        nodes = np.arange(nbounds[c], nbounds[c + 1])
        assert len(nodes) <= NLOC
        order = nodes[np.argsort(-indeg[nodes], kind="stable")]
        loads = np.zeros(NTILES, np.int64)
        counts = np.zeros(NTILES, np.int64)
        for nd in order:
            free = np.nonzero(counts < 128)[0]
            tgt = free[np.argmin(loads[free])]
            tile_of_node[nd] = tgt
            slotpos_of_node[nd] = counts[tgt]
            glob2slot[nd] = c * NLOC + tgt * 128 + counts[tgt]
            counts[tgt] += 1
            loads[tgt] += indeg[nd]
        assert loads.max() <= TPE

    edge_core = core_of_node[dst]
    idx_l, dp_l, gid_l = [], [], []
    for c in range(NCORES):
        eids = np.nonzero(edge_core == c)[0]
        assert len(eids) <= ELOC
        src_slot = np.zeros(ELOC, np.int64)
        dst_pos = np.full(ELOC, -1, np.int64)
        et = tile_of_node[dst[eids]]
        for t in range(NTILES):
            sel = eids[et == t]
            assert len(sel) <= TPE
            b = t * TPE
            src_slot[b : b + len(sel)] = glob2slot[src[sel]]
            dst_pos[b : b + len(sel)] = slotpos_of_node[dst[sel]]
        idx_l.append(np.ascontiguousarray(
            src_slot.reshape(NCH, 128).T).astype(np.int32))
        dp_l.append(np.ascontiguousarray(
            dst_pos.reshape(NCH, 128).T).astype(np.int32))
        gid = np.full((128, NTILES), -1, np.int64)
        nodes = np.arange(nbounds[c], nbounds[c + 1])
        gid[slotpos_of_node[nodes], tile_of_node[nodes]] = \
            graph_ids[nodes] - c * GPC
        gid_l.append(gid.astype(np.int32))

    x1 = np.zeros((NSTAR, F_IN), np.float32)
    x1[glob2slot] = node_feat
    x1T_l = [np.ascontiguousarray(x1[c * NLOC:(c + 1) * NLOC].T).astype(BF16)
             for c in range(NCORES)]
    return dict(idx=idx_l, dp=dp_l, gid=gid_l, x1T=x1T_l)


def fold_weights(W, al, ar):
    """[Din, Dout] (+attn vecs) -> [Din, Dout+8] = [W | W@al_h | W@ar_h]."""
    Din, D = W.shape
    Hh, O = al.shape
    Wl = np.einsum("iho,ho->ih", W.reshape(Din, Hh, O), al)
    Wr = np.einsum("iho,ho->ih", W.reshape(Din, Hh, O), ar)
    return np.concatenate([W, Wl, Wr], 1)


def wstack_flat(Waug):
    """[Din, C] -> flat (p, k, c)-major [128 * Din/128 * C]."""
    Din, C = Waug.shape
    return np.ascontiguousarray(
        Waug.reshape(Din // 128, 128, C).transpose(1, 0, 2)).reshape(-1)


def pack_weights(Ws, als, ars, bs, Wg1, bg1, Wg2, bg2, Wmu, bmu, Wlv, blv):
    bfb = np.zeros(TOTB, BF16)
    def putb(nm, a):
        a = np.asarray(a, np.float32).reshape(-1)
        bfb[OFFB[nm]:OFFB[nm] + len(a)] = a.astype(BF16)
    for i in range(3):
        putb(f"Wa{i+1}", wstack_flat(fold_weights(Ws[i], als[i], ars[i])))
        putb(f"b{i+1}", bs[i])
    putb("Wg1", wstack_flat(np.asarray(Wg1, np.float32)))
    putb("Wg2", Wg2)
    f32b = np.zeros(TOTF, np.float32)
    def putf(nm, a):
        a = np.asarray(a, np.float32).reshape(-1)
        f32b[OFFF[nm]:OFFF[nm] + len(a)] = a
    putf("Wmu", wstack_flat(np.asarray(Wmu, np.float32)))
    putf("Wlv", wstack_flat(np.asarray(Wlv, np.float32)))
    putf("bg1", bg1)
    putf("bg2", np.full(128, np.asarray(bg2, np.float32).reshape(-1)[0]))
    putf("bmu", bmu)
    putf("blv", blv)
    return bfb.reshape(NCORES, PB), f32b.reshape(NCORES, PF)


# ------------------------------------------------------------------ builder
def build_fused():
    import concourse.bass as bass
    import concourse.tile as tile
    from concourse import bacc, mybir

    bf = mybir.dt.bfloat16
    f32 = mybir.dt.float32
    i32 = mybir.dt.int32
    AF = mybir.ActivationFunctionType
    ALU = mybir.AluOpType
    RG = [list(range(NCORES))]

    nc = bacc.Bacc("TRN2", target_bir_lowering=False, debug=False,
                   num_devices=NCORES)
    x1T = nc.dram_tensor("x1T", [128, NLOC], bf, kind="ExternalInput").ap()
    idx = nc.dram_tensor("idx", [128, NCH], i32, kind="ExternalInput").ap()
    dp = nc.dram_tensor("dp", [128, NCH], i32, kind="ExternalInput").ap()
    gid = nc.dram_tensor("gid", [128, NTILES], i32, kind="ExternalInput").ap()
    wbf = nc.dram_tensor("wbf", [PB], bf, kind="ExternalInput").ap()
    wf32 = nc.dram_tensor("wf32", [PF], f32, kind="ExternalInput").ap()
    mulv = nc.dram_tensor("mulv", [GPAD, 256], f32, kind="ExternalOutput").ap()

    wbl = nc.dram_tensor("wbl", [PB], bf, kind="Internal").ap()
    wbs = nc.dram_tensor("wbs", [TOTB], bf, kind="Internal",
                         addr_space="Shared").ap()
    wfl = nc.dram_tensor("wfl", [PF], f32, kind="Internal").ap()
    wfs = nc.dram_tensor("wfs", [TOTF], f32, kind="Internal",
                         addr_space="Shared").ap()
    flocs, ftabs = [], []
    for l in range(3):
        flocs.append(nc.dram_tensor(f"floc{l}", [NLOC, ROWS[l]], bf,
                                    kind="Internal").ap())
        ftabs.append(nc.dram_tensor(f"ftab{l}", [NSTAR, ROWS[l]], bf,
                                    kind="Internal", addr_space="Shared").ap())

    with tile.TileContext(nc) as tc:
        with tc.tile_pool(name="cst", bufs=1) as cp, \
             tc.tile_pool(name="xper", bufs=1) as xp, \
             tc.tile_pool(name="trp", bufs=2, space="PSUM") as trp:
            # ---- stage + allgather weights
            nc.sync.dma_start(wbl[:], wbf[:])
            nc.sync.dma_start(wfl[:], wf32[:])
            nc.gpsimd.collective_compute(
                "AllGather", ALU.bypass, replica_groups=RG,
                ins=[wbl[:]], outs=[wbs[:]])
            nc.gpsimd.collective_compute(
                "AllGather", ALU.bypass, replica_groups=RG,
                ins=[wfl[:]], outs=[wfs[:]])

            def segb(nm, shape):
                n = int(np.prod(shape))
                a = wbs[OFFB[nm]:OFFB[nm] + n]
                if len(shape) == 3:
                    return a.rearrange("(p k c) -> p k c", p=shape[0],
                                       k=shape[1], c=shape[2])
                return a.rearrange("(a c) -> a c", a=shape[0], c=shape[1])

            def segf(nm, shape):
                n = int(np.prod(shape))
                a = wfs[OFFF[nm]:OFFF[nm] + n]
                if len(shape) == 3:
                    return a.rearrange("(p k c) -> p k c", p=shape[0],
                                       k=shape[1], c=shape[2])
                return a.rearrange("(a c) -> a c", a=shape[0], c=shape[1])

            # ---- constants
            iota_row = cp.tile([128, 128], i32, tag="io_r")
            nc.gpsimd.iota(iota_row[:], pattern=[[1, 128]], base=0,
                           channel_multiplier=0)
            iota_col = cp.tile([128, 128], i32, tag="io_c")
            nc.gpsimd.iota(iota_col[:], pattern=[[0, 128]], base=0,
                           channel_multiplier=1)
            iota16 = iota_row[:].unsqueeze(1).broadcast_to([128, CPT, 128])
            ident = cp.tile([128, 128], bf, tag="ident")
            nc.vector.tensor_tensor(out=ident[:], in0=iota_row[:],
                                    in1=iota_col[:], op=ALU.is_equal)
            ident32 = cp.tile([32, 32], f32, tag="id32")
            nc.vector.tensor_tensor(out=ident32[:], in0=iota_row[0:32, 0:32],
                                    in1=iota_col[0:32, 0:32], op=ALU.is_equal)
            on1 = cp.tile([1, 128], bf, tag="on1")
            nc.vector.memset(on1[:], 1.0)
            on32 = cp.tile([1, 32], f32, tag="on32")
            nc.vector.memset(on32[:], 1.0)

            idxsb = cp.tile([128, NCH], i32, tag="idx")
            nc.sync.dma_start(idxsb[:], idx[:])
            dpsb = cp.tile([128, NCH], i32, tag="dp")
            nc.sync.dma_start(dpsb[:], dp[:])
            gidsb = cp.tile([128, NTILES], i32, tag="gid")
            nc.sync.dma_start(gidsb[:], gid[:])

            xtiles = {1: [], 2: [], 3: []}   # layer outputs [128, Dout]/tile

            # ================================================== layers
            for l, (Din, Dout) in enumerate(LAYERS):
                K, O, ROW = Din // 128, Dout // H, ROWS[l]
                fcks = _colchunks(Dout + 8)
                with tc.tile_pool(name=f"l{l}c", bufs=1) as lc:
                    ers = lc.tile([128, NTILES * 4], bf, tag="ers")
                    bsb = lc.tile([128, Dout], bf, tag="bsb")
                    Wa = lc.tile([128, K, Dout + 8], bf, tag="Wa")
                    nc.sync.dma_start(Wa[:], segb(f"Wa{l+1}",
                                                  [128, K, Dout + 8]))
                    brow = lc.tile([1, Dout], bf, tag="br")
                    nc.sync.dma_start(brow[:], segb(f"b{l+1}", [1, Dout]))
                    if l == 0:
                        x1Ts = lc.tile([128, NLOC], bf, tag="x1T")
                        nc.sync.dma_start(x1Ts[:], x1T[:])
                    # ---------- P phase: feat/el/er matmuls on own nodes
                    with tc.tile_pool(name=f"P{l}s", bufs=3) as sp, \
                         tc.tile_pool(name=f"P{l}p", bufs=1,
                                      space="PSUM") as pp:
                        bps = pp.tile([128, Dout], f32, tag="bps")
                        for (s, w) in _colchunks(Dout):
                            nc.tensor.matmul(out=bps[:, s:s + w],
                                             lhsT=on1[:],
                                             rhs=brow[:, s:s + w],
                                             start=True, stop=True)
                        nc.vector.tensor_copy(bsb[:], bps[:])
                        for t in range(NTILES):
                            pas = [pp.tile([128, w], f32, tag=f"pa{j}",
                                           name=f"pa{j}")
                                   for j, (s, w) in enumerate(fcks)]
                            for kc in range(K):
                                if l == 0:
                                    xt = x1Ts[:, t * 128:(t + 1) * 128]
                                else:
                                    ptr = trp.tile([128, 128], bf, tag="tr")
                                    nc.tensor.transpose(
                                        out=ptr[:],
                                        in_=xtiles[l][t][:, kc * 128:
                                                         (kc + 1) * 128],
                                        identity=ident[:])
                                    xts = sp.tile([128, 128], bf, tag="xt")
                                    nc.scalar.activation(xts[:], ptr[:],
                                                         AF.Copy)
                                    xt = xts[:]
                                for j, (s, w) in enumerate(fcks):
                                    nc.tensor.matmul(
                                        out=pas[j][:], lhsT=xt,
                                        rhs=Wa[:, kc, s:s + w],
                                        start=(kc == 0), stop=(kc == K - 1))
                            ft = sp.tile([128, ROW], bf, tag="ft")
                            for j, (s, w) in enumerate(fcks):
                                fw = min(s + w, Dout + 4) - s
                                if fw > 0:
                                    nc.vector.tensor_copy(ft[:, s:s + fw],
                                                          pas[j][:, 0:fw])
                            nc.vector.tensor_copy(
                                ers[:, t * 4:(t + 1) * 4],
                                pas[-1][:, fcks[-1][1] - 4:fcks[-1][1]])
                            nc.sync.dma_start(
                                flocs[l][t * 128:(t + 1) * 128, :], ft[:])
                    # ---------- AllGather feature table
                    nc.gpsimd.collective_compute(
                        "AllGather", ALU.bypass, replica_groups=RG,
                        ins=[flocs[l][:]], outs=[ftabs[l][:]])
                    # ---------- L phase: gather, edge softmax, aggregate
                    with tc.tile_pool(name=f"L{l}g", bufs=2) as gp, \
                         tc.tile_pool(name=f"L{l}o", bufs=2) as ohp, \
                         tc.tile_pool(name=f"L{l}t", bufs=3) as otp, \
                         tc.tile_pool(name=f"L{l}s", bufs=2) as sp, \
                         tc.tile_pool(name=f"L{l}p", bufs=1,
                                      space="PSUM") as pp:
                        for t in range(NTILES):
                            gtt = gp.tile([128, CPT * ROW], bf, tag="g")
                            oh16 = ohp.tile([128, CPT * 128], bf, tag="oh")
                            nc.vector.tensor_tensor(
                                out=oh16[:].rearrange("p (c e) -> p c e",
                                                      c=CPT, e=128),
                                in0=dpsb[:, t * CPT:(t + 1) * CPT]
                                    .to_broadcast([128, CPT, 128]),
                                in1=iota16, op=ALU.is_equal)
                            erps = pp.tile([128, CPT * 4], f32, tag="erps")
                            for c in range(CPT):
                                ch = t * CPT + c
                                nc.gpsimd.indirect_dma_start(
                                    out=gtt[:, c * ROW:(c + 1) * ROW],
                                    out_offset=None, in_=ftabs[l][:],
                                    in_offset=bass.IndirectOffsetOnAxis(
                                        ap=idxsb[:, ch:ch + 1], axis=0))
                                ptr = trp.tile([128, 128], bf, tag="tr")
                                nc.tensor.transpose(
                                    out=ptr[:],
                                    in_=oh16[:, c * 128:(c + 1) * 128],
                                    identity=ident[:])
                                oht = otp.tile([128, 128], bf, tag="oht")
                                nc.scalar.activation(oht[:], ptr[:], AF.Copy)
                                nc.tensor.matmul(
                                    out=erps[:, c * 4:(c + 1) * 4],
                                    lhsT=oht[:],
                                    rhs=ers[:, t * 4:(t + 1) * 4],
                                    start=True, stop=True)
                            g3 = gtt[:].rearrange("p (c r) -> p c r",
                                                  c=CPT, r=ROW)
                            zz = sp.tile([128, CPT * 4], f32, tag="zz")
                            nc.vector.tensor_add(
                                zz[:].rearrange("p (c h) -> p c h",
                                                c=CPT, h=4),
                                g3[:, :, Dout:Dout + 4],
                                erps[:].rearrange("p (c h) -> p c h",
                                                  c=CPT, h=4))
                            za = sp.tile([128, CPT * 4], f32, tag="za")
                            nc.vector.scalar_tensor_tensor(
                                out=za[:], in0=zz[:], scalar=NEG_SLOPE,
                                in1=zz[:], op0=ALU.mult, op1=ALU.max)
                            ee = sp.tile([128, CPT * 4], bf, tag="ee")
                            nc.scalar.activation(ee[:], za[:], AF.Exp)
                            # ee into the row tail (feeds the den columns)
                            nc.vector.tensor_copy(
                                g3[:, :, Dout + 4:Dout + 8],
                                ee[:].rearrange("p (c h) -> p c h",
                                                c=CPT, h=4))
                            rstps = [pp.tile([128, w], f32, tag=f"rst{j}",
                                             name=f"rst{j}")
                                     for j, (s, w) in enumerate(fcks)]
                            for c in range(CPT):
                                e4 = ee[:, c * 4:(c + 1) * 4]
                                gslc = gtt[:, c * ROW:c * ROW + Dout] \
                                    .rearrange("p (h o) -> p h o", h=H, o=O)
                                nc.vector.scalar_tensor_tensor(
                                    out=gslc, in0=gslc, scalar=1.0,
                                    in1=e4.to_broadcast([128, H, O]),
                                    op0=ALU.mult, op1=ALU.mult)
                                for j, (s, w) in enumerate(fcks):
                                    nc.tensor.matmul(
                                        out=rstps[j][:],
                                        lhsT=oh16[:, c * 128:(c + 1) * 128],
                                        rhs=gtt[:, c * ROW + s:
                                                c * ROW + s + w],
                                        start=(c == 0), stop=(c == CPT - 1))
                            wl = fcks[-1][1]
                            dcl = sp.tile([128, 4], f32, tag="dcl")
                            nc.vector.tensor_scalar_max(
                                dcl[:], rstps[-1][:, wl - 4:wl], 1e-9)
                            rec = sp.tile([128, 4], f32, tag="rec")
                            nc.vector.reciprocal(rec[:], dcl[:])
                            y = sp.tile([128, Dout], f32, tag="y", bufs=1)
                            for h in range(H):
                                j = (h * O) // 512
                                s0 = (h * O) % 512
                                nc.vector.scalar_tensor_tensor(
                                    out=y[:, h * O:(h + 1) * O],
                                    in0=rstps[j][:, s0:s0 + O],
                                    scalar=rec[:, h:h + 1],
                                    in1=bsb[:, h * O:(h + 1) * O],
                                    op0=ALU.mult, op1=ALU.add)
                            mn = sp.tile([128, Dout], f32, tag="mn", bufs=1)
                            nc.vector.tensor_scalar_min(mn[:], y[:], 0.0)
                            nc.scalar.activation(mn[:], mn[:], AF.Exp)
                            nc.vector.scalar_tensor_tensor(
                                out=y[:], in0=y[:], scalar=0.0, in1=mn[:],
                                op0=ALU.max, op1=ALU.add)
                            xo = xp.tile([128, Dout], bf, tag=f"xo{l}_{t}",
                                         name=f"xo{l}_{t}")
                            nc.vector.tensor_scalar_add(xo[:], y[:], -1.0)
                            xtiles[l + 1].append(xo)

            # ================================================== pooling
            h3 = xtiles[3]
            with tc.tile_pool(name="pc", bufs=1) as pc, \
                 tc.tile_pool(name="ps", bufs=3) as sp:
                Wg1s = pc.tile([128, 8, 128], bf, tag="Wg1")
                nc.sync.dma_start(Wg1s[:], segb("Wg1", [128, 8, 128]))
                Wg2c = pc.tile([128, 1], bf, tag="Wg2")
                nc.sync.dma_start(Wg2c[:], segb("Wg2", [128, 1]))
                Wmus = pc.tile([128, 8, 128], f32, tag="Wmu")
                nc.sync.dma_start(Wmus[:], segf("Wmu", [128, 8, 128]))
                Wlvs = pc.tile([128, 8, 128], f32, tag="Wlv")
                nc.sync.dma_start(Wlvs[:], segf("Wlv", [128, 8, 128]))
                bg1c = pc.tile([128, 1], f32, tag="bg1")
                nc.sync.dma_start(bg1c[:], segf("bg1", [128, 1]))
                bg2r = pc.tile([128, 1], f32, tag="bg2")
                nc.sync.dma_start(bg2r[:], segf("bg2", [128, 1]))
                bmur = pc.tile([1, 128], f32, tag="bmu")
                nc.sync.dma_start(bmur[:], segf("bmu", [1, 128]))
                blvr = pc.tile([1, 128], f32, tag="blv")
                nc.sync.dma_start(blvr[:], segf("blv", [1, 128]))
                h3Ts = pc.tile([128, 8 * NLOC], bf, tag="h3T")
                relu1 = pc.tile([128, NLOC], bf, tag="relu1")
                eg = pc.tile([128, NTILES], bf, tag="eg")
                with tc.tile_pool(name="ppa", bufs=1, space="PSUM") as pa:
                    for t in range(NTILES):
                        for kc in range(8):
                            ptr = trp.tile([128, 128], bf, tag="tr")
                            nc.tensor.transpose(
                                out=ptr[:],
                                in_=h3[t][:, kc * 128:(kc + 1) * 128],
                                identity=ident[:])
                            nc.scalar.activation(
                                h3Ts[:, kc * NLOC + t * 128:
                                     kc * NLOC + (t + 1) * 128],
                                ptr[:], AF.Copy)
                    nwin = (NLOC + 511) // 512
                    for w in range(nwin):
                        s = w * 512
                        ww = min(512, NLOC - s)
                        g1p = pa.tile([128, 512], f32, tag="g1", bufs=2)
                        for kc in range(8):
                            nc.tensor.matmul(
                                out=g1p[:, :ww], lhsT=Wg1s[:, kc, :],
                                rhs=h3Ts[:, kc * NLOC + s:kc * NLOC + s + ww],
                                start=(kc == 0), stop=(kc == 7))
                        nc.scalar.activation(relu1[:, s:s + ww], g1p[:, :ww],
                                             AF.Relu, bias=bg1c[:])
                    gps = pa.tile([128, 32], f32, tag="g2")
                    for t in range(NTILES):
                        nc.tensor.matmul(out=gps[:, t:t + 1],
                                         lhsT=relu1[:, t * 128:(t + 1) * 128],
                                         rhs=Wg2c[:], start=True, stop=True)
                    nc.scalar.activation(eg[:], gps[:, :NTILES], AF.Exp,
                                         bias=bg2r[:])
                with tc.tile_pool(name="ppb", bufs=1, space="PSUM") as pb:
                    gd = pb.tile([GPAD, 1], f32, tag="gd")
                    geps = [pb.tile([GPAD, 512], f32, tag=f"ge{j}",
                                    name=f"geps{j}") for j in range(2)]
                    for t in range(NTILES):
                        goh = sp.tile([128, GPAD], bf, tag="goh")
                        nc.vector.tensor_tensor(
                            out=goh[:],
                            in0=gidsb[:, t:t + 1].to_broadcast([128, GPAD]),
                            in1=iota_row[:, 0:GPAD], op=ALU.is_equal)
                        nc.tensor.matmul(out=gd[:], lhsT=goh[:],
                                         rhs=eg[:, t:t + 1],
                                         start=(t == 0),
                                         stop=(t == NTILES - 1))
                        goha = sp.tile([128, GPAD], bf, tag="goha")
                        nc.vector.tensor_mul(
                            goha[:], goh[:],
                            eg[:, t:t + 1].to_broadcast([128, GPAD]))
                        for j in range(2):
                            nc.tensor.matmul(
                                out=geps[j][:], lhsT=goha[:],
                                rhs=h3[t][:, j * 512:(j + 1) * 512],
                                start=(t == 0), stop=(t == NTILES - 1))
                    gdc = sp.tile([GPAD, 1], f32, tag="gdc")
                    nc.vector.tensor_scalar_max(gdc[:], gd[:], 1e-9)
                    grc = sp.tile([GPAD, 1], f32, tag="grc")
                    nc.vector.reciprocal(grc[:], gdc[:])
                    zge = sp.tile([GPAD, D3], f32, tag="zge", bufs=1)
                    nc.vector.memset(zge[:], 0.0)
                    ge = sp.tile([GPAD, D3], f32, tag="ge", bufs=1)
                    for j in range(2):
                        nc.vector.scalar_tensor_tensor(
                            out=ge[:, j * 512:(j + 1) * 512], in0=geps[j][:],
                            scalar=grc[:, 0:1],
                            in1=zge[:, j * 512:(j + 1) * 512],
                            op0=ALU.mult, op1=ALU.add)
                    geT = pc.tile([128, 8 * GPAD], f32, tag="geT")
                    for kc in range(8):
                        pst = pb.tile([128, GPAD], f32, tag="pst", bufs=1)
                        nc.tensor.transpose(out=pst[:],
                                            in_=ge[:, kc * 128:(kc + 1) * 128],
                                            identity=ident32[:])
                        nc.vector.tensor_copy(
                            geT[:, kc * GPAD:(kc + 1) * GPAD], pst[:])
                    for oi, (Wt, bt) in enumerate([(Wmus, bmur),
                                                   (Wlvs, blvr)]):
                        mps = pb.tile([GPAD, 128], f32, tag="mps", bufs=1)
                        for kc in range(8):
                            nc.tensor.matmul(
                                out=mps[:],
                                lhsT=geT[:, kc * GPAD:(kc + 1) * GPAD],
                                rhs=Wt[:, kc, :],
                                start=(kc == 0), stop=False)
                        nc.tensor.matmul(out=mps[:], lhsT=on32[:], rhs=bt[:],
                                         start=False, stop=True)
                        mo = sp.tile([GPAD, 128], f32, tag="mo")
                        nc.vector.tensor_copy(mo[:], mps[:])
                        nc.sync.dma_start(
                            mulv[:, oi * 128:(oi + 1) * 128], mo[:])
    nc.compile()
    return nc


_BUILD_CACHE = {}


def _get(key, fn):
    if key not in _BUILD_CACHE:
        _BUILD_CACHE[key] = fn()
    return _BUILD_CACHE[key]


def _make_runner(nc):
    """Build the PJRT executable for ``nc`` once and reuse it across calls —
    the stock run_bass_kernel_spmd re-jits (and re-lowers through XLA) on
    every invocation."""
    import jax
    import numpy as _np
    from jax.sharding import Mesh, PartitionSpec
    from jax.experimental.shard_map import shard_map
    from concourse import bass2jax, mybir

    bass2jax.install_neuronx_cc_hook()
    partition_name = (nc.partition_id_tensor.name
                      if nc.partition_id_tensor else None)
    in_names, out_names, out_avals, zero_outs = [], [], [], []
    for alloc in nc.m.functions[0].allocations:
        if not isinstance(alloc, mybir.MemoryLocationSet):
            continue
        name = alloc.memorylocations[0].name
        if alloc.kind == "ExternalInput":
            if name != partition_name:
                in_names.append(name)
        elif alloc.kind == "ExternalOutput":
            shape = tuple(alloc.tensor_shape)
            dtype = mybir.dt.np(alloc.dtype)
            out_names.append(name)
            out_avals.append(jax.core.ShapedArray(shape, dtype))
            zero_outs.append(_np.zeros(shape, dtype))
    n_params, n_outs = len(in_names), len(out_avals)
    in_names = in_names + out_names
    if partition_name is not None:
        in_names.append(partition_name)
    donate = tuple(range(n_params, n_params + n_outs))

    def _body(*args):
        operands = list(args)
        if partition_name is not None:
            operands.append(bass2jax.partition_id_tensor())
        outs = bass2jax._bass_exec_p.bind(
            *operands, out_avals=tuple(out_avals), in_names=tuple(in_names),
            out_names=tuple(out_names), lowering_input_output_aliases=(),
            sim_require_finite=True, sim_require_nnan=True, nc=nc)
        return tuple(outs)

    devices = jax.devices()[:NCORES]
    mesh = Mesh(np.asarray(devices), ("core",))
    # No donation: the kernel writes every element of its outputs, so the
    # zero "output operand" buffers stay valid across calls and can live on
    # device permanently (donating them would invalidate the cached copy and
    # force a per-call host->device upload).
    sharded = jax.jit(
        shard_map(_body, mesh=mesh,
                  in_specs=(PartitionSpec("core"),) * (n_params + n_outs),
                  out_specs=(PartitionSpec("core"),) * n_outs,
                  check_rep=False),
        keep_unused=True)
    from jax.sharding import NamedSharding
    dev_zeros = [
        jax.device_put(np.zeros((NCORES * z.shape[0], *z.shape[1:]), z.dtype),
                       NamedSharding(mesh, PartitionSpec("core")))
        for z in zero_outs]
    return dict(fn=sharded, in_names=in_names[:n_params],
                out_names=out_names, out_avals=out_avals,
                zero_outs=zero_outs, dev_zeros=dev_zeros, mesh=mesh)


_DEVPUT_CACHE = {}


def _stage_input(nm, concat, mesh):
    """Upload ``concat`` sharded by core, reusing the device copy when the
    content is unchanged since the previous call (weights / graph structure
    normally are).  Content is verified host-side with memcmp."""
    import jax
    from jax.sharding import NamedSharding, PartitionSpec
    ent = _DEVPUT_CACHE.get(nm)
    if ent is not None and ent[0].shape == concat.shape \
            and ent[0].dtype == concat.dtype and np.array_equal(ent[0], concat):
        return ent[1]
    arr = jax.device_put(
        concat, NamedSharding(mesh, PartitionSpec("core")))
    _DEVPUT_CACHE[nm] = (concat, arr)
    return arr


_FETCH_POOL = None


def _fetch_np(arrs):
    """Device→host fetch with one thread per shard — the per-shard tunnel
    round trips dominate small-output D2H when done serially."""
    global _FETCH_POOL
    import concurrent.futures as cf
    if _FETCH_POOL is None:
        _FETCH_POOL = cf.ThreadPoolExecutor(16)
    outs, jobs = [], []
    for a in arrs:
        res = np.empty(a.shape, a.dtype)
        outs.append(res)
        for sh in a.addressable_shards:
            jobs.append(_FETCH_POOL.submit(
                lambda r, s: r.__setitem__(s.index, np.asarray(s.data)),
                res, sh))
    for j in jobs:
        j.result()
    return outs


def _run(nc, in_maps):
    import time as _time
    dbg = bool(__import__("os").environ.get("KERNEL_DEBUG_TIMING"))
    r = _get(("RUNNER", id(nc)), lambda: _make_runner(nc))
    t0 = _time.time()
    concat_in = [
        _stage_input(nm,
                     np.concatenate([np.asarray(m[nm]) for m in in_maps], 0),
                     r["mesh"])
        for nm in r["in_names"]]
    t1 = _time.time()
    out_arrs = r["fn"](*concat_in, *r["dev_zeros"])
    out_np = _fetch_np(out_arrs)
    t2 = _time.time()
    if dbg:
        print(f"[timing] stage-in {t1-t0:.3f}s  dispatch+exec+D2H "
              f"{t2-t1:.3f}s")
    return [
        {nm: out_np[i].reshape(NCORES, *r["out_avals"][i].shape)[c]
         for i, nm in enumerate(r["out_names"])}
        for c in range(NCORES)
    ]


# ------------------------------------------------------------------ main entry
def kernel(node_feat, src, dst, graph_ids,
           W1, al1, ar1, b1, W2, al2, ar2, b2, W3, al3, ar3, b3,
           Wg1, bg1, Wg2, bg2, Wmu, bmu, Wlv, blv):
    prep = host_prep(node_feat, src, dst, graph_ids)
    wb, wf = pack_weights(
        [np.asarray(W1, np.float32), np.asarray(W2, np.float32),
         np.asarray(W3, np.float32)],
        [np.asarray(al1, np.float32), np.asarray(al2, np.float32),
         np.asarray(al3, np.float32)],
        [np.asarray(ar1, np.float32), np.asarray(ar2, np.float32),
         np.asarray(ar3, np.float32)],
        [np.asarray(b1, np.float32), np.asarray(b2, np.float32),
         np.asarray(b3, np.float32)],
        Wg1, bg1, Wg2, bg2, Wmu, bmu, Wlv, blv)
    ncF = _get(("FUSED",), build_fused)
    in_maps = [dict(x1T=prep["x1T"][c], idx=prep["idx"][c], dp=prep["dp"][c],
                    gid=prep["gid"][c], wbf=wb[c], wf32=wf[c])
               for c in range(NCORES)]
    res = _run(ncF, in_maps)
    mu = np.concatenate([res[c]["mulv"][:GPC, :128] for c in range(NCORES)], 0)
    lv = np.concatenate([res[c]["mulv"][:GPC, 128:] for c in range(NCORES)], 0)
    return np.asarray(mu, np.float32), np.asarray(lv, np.float32)
